# Initial kernel scaffold
#
# Trainium2 Bass kernel for MC-Stereo-like iterative disparity refinement.
# Self-contained: hardcodes shapes; shards H=96 across 8 NeuronCores (12 own
# rows + 6-row halo), refreshes halos between iterations with an AllGather.
import sys, os
sys.path.insert(0, '/opt/trn_rl_repo')
import numpy as np
import ml_dtypes

import concourse.bass as bass
import concourse.bacc as bacc
import concourse.mybir as mybir
import concourse.tile as tile
from concourse import bass_utils

F32 = mybir.dt.float32
F32R = mybir.dt.float32r
BF16 = mybir.dt.bfloat16
I32 = mybir.dt.int32
U32 = mybir.dt.uint32
OP = mybir.AluOpType
AF = mybir.ActivationFunctionType

H, W, C, HID = 96, 160, 256, 128
K, RT, DMAX, FACTOR = 3, 12, 48, 4
S = 2 * RT + 1            # 25
NC_ = 8                   # cores
OWN, HALO, SLAB = 12, 6, 24
NPX = SLAB * W            # 3840
NT = NPX // 128           # 30 px tiles
WP = W + 2                # 162 padded width
VPAD, VW = 49, 184        # vol_pad: data at cols [49,97)
WIN = 52                  # gathered window width
# barrel stage widths (src width consumed per stage, high bit first)
BITS = [64, 32, 16, 8, 4, 2, 1]
BWID = {64: 115, 32: 83, 16: 67, 8: 59, 4: 55, 2: 53, 1: 52}
ITERS = 3

_cache = {}
_last_res = None


def _conv_shift_rhs(slab_ap, r0, dy, dx, nrows):
    # rhs AP [K, nrows, 160] for conv output rows [r0, r0+nrows) at shift (dy,dx)
    return slab_ap[:, r0 + dy - 1:r0 + dy - 1 + nrows, dx:dx + W]


def build():
    nc = bacc.Bacc("TRN2", target_bir_lowering=False, debug=False,
                   num_devices=NC_)

    def inp(name, shape, dtype=F32):
        return nc.dram_tensor(name, list(shape), dtype, kind="ExternalInput")

    F1 = inp("F1", [2, 128, SLAB, W]); F2 = inp("F2", [2, 128, SLAB, W])
    NET0 = inp("NET0", [HID, SLAB, W])
    CZS = inp("CZS", [HID, 20, W]); CQS = inp("CQS", [HID, 20, W]); CRS = inp("CRS", [HID, 20, W])
    WENC0 = inp("WENC0", [128, 9, 128], BF16); WENC1 = inp("WENC1", [25, 9, 128], BF16)
    WZ0 = inp("WZ0", [128, 9, 128], BF16); WZ1 = inp("WZ1", [128, 9, 128], BF16)
    WR0 = inp("WR0", [128, 9, 128], BF16); WR1 = inp("WR1", [128, 9, 128], BF16)
    WQ0 = inp("WQ0", [128, 9, 128], BF16); WQ1 = inp("WQ1", [128, 9, 128], BF16)
    WD1 = inp("WD1", [128, 9, 128]); WD2 = inp("WD2", [128, 9, 75])
    WM0 = inp("WM0", [128, 9, 128], BF16); WM1 = inp("WM1", [128, 9, 16], BF16)
    BENC = inp("BENC", [128, 1]); BD1 = inp("BD1", [128, 1]); BD2 = inp("BD2", [75, 1])
    BZ = inp("BZ", [128, 1]); BR = inp("BR", [128, 1]); BQ = inp("BQ", [128, 1])
    BME0 = inp("BME0", [128, 1]); BME1 = inp("BME1", [16, 1])
    DELTAS = inp("DELTAS", [128, 25])
    MROW = inp("MROW", [128, SLAB])
    PXM = inp("PXM", [128, NT])
    W0M = inp("W0M", [128, 16]); W159M = inp("W159M", [128, 16])
    AGNT = inp("AGNT", [128, 8]); AGNB = inp("AGNB", [128, 8])
    MDTT = inp("MDTT", [128, 64]); MDTOT = inp("MDTOT", [128, 8])
    MDTB = inp("MDTB", [128, 64]); MDTOB = inp("MDTOB", [128, 8])

    OUT = nc.dram_tensor("OUT", [ITERS, 4 * OWN, 4 * W], F32, kind="ExternalOutput")
    DBGDT = nc.dram_tensor("DBGDT", [128, NT, 3], F32, kind="ExternalOutput")
    DBGNET = nc.dram_tensor("DBGNET", [128, SLAB, WP], F32, kind="ExternalOutput")

    with tile.TileContext(nc) as tc:
        with (
            tc.tile_pool(name="persist", bufs=1) as pp,
            tc.tile_pool(name="work", bufs=2) as wk,
            tc.tile_pool(name="barrel", bufs=4) as bp,
            tc.tile_pool(name="stream", bufs=3) as st,
            tc.tile_pool(name="stgp", bufs=1) as stp,
            tc.tile_pool(name="agp", bufs=1) as agp,
            tc.tile_pool(name="psc", bufs=4, space="PSUM") as psc,
            tc.tile_pool(name="pst", bufs=4, space="PSUM") as pst,
            tc.tile_pool(name="dram", bufs=1, space="DRAM") as dr,
        ):
            # ---------- load persistent constants / weights ----------
            def load(t_dram, shape, dtype, name):
                t = pp.tile(list(shape), dtype, name=name, tag=name)
                if dtype in (BF16, F32R) and t_dram.dtype != dtype:
                    nc.gpsimd.dma_start(t[:], t_dram.ap())
                else:
                    nc.sync.dma_start(t[:], t_dram.ap())
                return t

            wenc0 = load(WENC0, (128, 9, 128), BF16, "wenc0")
            wenc1 = load(WENC1, (25, 9, 128), BF16, "wenc1")
            wz0 = load(WZ0, (128, 9, 128), BF16, "wz0"); wz1 = load(WZ1, (128, 9, 128), BF16, "wz1")
            wr0 = load(WR0, (128, 9, 128), BF16, "wr0"); wr1 = load(WR1, (128, 9, 128), BF16, "wr1")
            wq0 = load(WQ0, (128, 9, 128), BF16, "wq0"); wq1 = load(WQ1, (128, 9, 128), BF16, "wq1")
            wd1 = load(WD1, (128, 9, 128), F32R, "wd1"); wd2 = load(WD2, (128, 9, 75), F32R, "wd2")
            wm0 = load(WM0, (128, 9, 128), BF16, "wm0"); wm1 = load(WM1, (128, 9, 16), BF16, "wm1")
            benc = load(BENC, (128, 1), F32, "benc")
            bd1 = load(BD1, (128, 1), F32, "bd1"); bd2 = load(BD2, (75, 1), F32, "bd2")
            bz = load(BZ, (128, 1), F32, "bz"); br = load(BR, (128, 1), F32, "br"); bq = load(BQ, (128, 1), F32, "bq")
            bme0 = load(BME0, (128, 1), F32, "bme0"); bme1 = load(BME1, (16, 1), F32, "bme1")
            deltas = load(DELTAS, (128, 25), F32, "deltas")
            mrow = load(MROW, (128, SLAB), F32, "mrow")
            pxm = load(PXM, (128, NT), F32, "pxm")
            w0m = load(W0M, (128, 16), F32, "w0m"); w159m = load(W159M, (128, 16), F32, "w159m")
            agnt = load(AGNT, (128, 8), F32, "agnt"); agnb = load(AGNB, (128, 8), F32, "agnb")
            mdtt = load(MDTT, (128, 64), F32, "mdtt"); mdtot = load(MDTOT, (128, 8), F32, "mdtot")
            mdtb = load(MDTB, (128, 64), F32, "mdtb"); mdtob = load(MDTOB, (128, 8), F32, "mdtob")

            ident = pp.tile([128, 128], BF16, name="ident", tag="ident")
            ones128 = pp.tile([128, 128], BF16, name="ones128", tag="ones128")
            nc.vector.memset(ones128[:], 1.0)
            nc.gpsimd.affine_select(ident[:], ones128[:], pattern=[[-1, 128]], base=0,
                                    channel_multiplier=1, compare_op=OP.is_equal, fill=0.0)
            onesf = pp.tile([128, 128], F32, name="onesf", tag="onesf")
            nc.vector.memset(onesf[:], 1.0)
            identf = pp.tile([128, 128], F32, name="identf", tag="identf")
            nc.gpsimd.affine_select(identf[:], onesf[:], pattern=[[-1, 128]], base=0,
                                    channel_multiplier=1, compare_op=OP.is_equal, fill=0.0)


            # ---------- persistent state ----------
            volp = pp.tile([128, NT, VW], F32, name="volp", tag="volp")
            nc.vector.memset(volp[:], 0.0)
            net = pp.tile([128, SLAB, WP], F32R, name="net", tag="net")
            netbf = pp.tile([128, SLAB, WP], BF16, name="netbf", tag="netbf")
            dtA = pp.tile([128, NT, 3], F32, name="dtA", tag="dtA")
            dtB = pp.tile([128, NT, 3], F32, name="dtB", tag="dtB")

            # ---------- Phase A: correlation volume ----------
            Rg = dr.tile([SLAB, 160, 208], F32)  # reversed gram rows
            zk = wk.tile([128, 48], F32, name="zk", tag="zk")
            nc.vector.memset(zk[:], 0.0)
            for b in range(30):
                nc.sync.dma_start(
                    bass.AP(tensor=Rg[:].tensor, offset=b * 128 * 208 + 160,
                            ap=[[208, 128], [1, 48]]), zk[:])
            for r in range(SLAB):
                f1r = [st.tile([128, W], F32, name=f"f1r{c}", tag="f1r") for c in range(2)]
                f2r = [st.tile([128, W], F32, name=f"f2r{c}", tag="f2r") for c in range(2)]
                for c in range(2):
                    nc.sync.dma_start(f1r[c][:], F1.ap()[c, :, r, :])
                    nc.sync.dma_start(f2r[c][:], F2.ap()[c, :, r, :])
                for (w0, m) in ((0, 128), (128, 32)):
                    pg = pst.tile([128, W], F32, name="pg", tag="pstr")
                    for c in range(2):
                        rev = bass.AP(tensor=f2r[c][:].tensor,
                                      offset=f2r[c][:].offset + 159,
                                      ap=[list(f2r[c][:].ap[0]), [-1, W]])
                        nc.tensor.matmul(pg[:m, :], f1r[c][:, w0:w0 + m], rev,
                                         start=(c == 0), stop=(c == 1))
                    gs = wk.tile([128, W], F32, name="gs", tag="gs")
                    nc.scalar.activation(gs[:m, :], pg[:m, :], AF.Copy, scale=1.0 / 16.0)
                    nc.sync.dma_start(Rg[:][r, w0:w0 + m, 0:160], gs[:m, :])
            # diagonal extraction -> volp[:, t, VPAD:VPAD+48]
            rgf = Rg[:].flatten()
            for r in range(SLAB):
                p0 = r * W
                p1 = p0 + W
                while p0 < p1:
                    t = p0 // 128
                    run = min(p1, (t + 1) * 128) - p0
                    w = p0 - r * W
                    src = bass.AP(tensor=rgf.tensor, offset=r * 33280 + w * 207 + 159,
                                  ap=[[207, run], [1, 48]])
                    nc.sync.dma_start(volp[p0 - 128 * t:p0 - 128 * t + run, t, VPAD:VPAD + 48], src)
                    p0 += run
            # initial top-3 (descending) of vol, masked by in-image
            for t in range(NT):
                tv = wk.tile([128, 8], F32, name="tv", tag="tv")
                ti = wk.tile([128, 8], U32, name="ti", tag="ti")
                nc.vector.max(tv[:], volp[:, t, VPAD:VPAD + 48])
                nc.vector.max_index(ti[:], tv[:], volp[:, t, VPAD:VPAD + 48])
                tif = wk.tile([128, 3], F32, name="tif", tag="tif")
                nc.vector.tensor_copy(tif[:], ti[:, 0:3].bitcast(I32))
                nc.vector.tensor_scalar(dtA[:, t, :], tif[:], pxm[:, t:t + 1], None, op0=OP.mult)
            # net = tanh(net0); pads zero
            nc.vector.memset(net[:].bitcast(F32), 0.0)
            nc.gpsimd.dma_start(net[:, :, 1:161], NET0.ap())
            nc.scalar.activation(net[:, :, 1:161], net[:, :, 1:161], AF.Tanh)
            nc.vector.tensor_copy(netbf[:], net[:])

            dram_dispf = dr.tile([4224], F32)
            zk2 = wk.tile([128, 33], F32, name="zk2", tag="zk2")
            nc.vector.memset(zk2[:], 0.0)
            nc.sync.dma_start(
                bass.AP(tensor=dram_dispf[:].tensor, offset=0, ap=[[33, 128], [1, 33]]), zk2[:])
            agins, agouts = [], []
            for _it in range(ITERS - 1):
                _ai = dr.tile([128, 1968], F32, name=f"agin{_it}", tag=f"agin{_it}")
                _ao = dr.tile([NC_ * 128, 1968], F32, addr_space="Shared", name=f"agout{_it}", tag=f"agout{_it}")
                agins.append(_ai); agouts.append(_ao)

            dt_cur, dt_nxt = dtA, dtB
            for it in range(ITERS):
              with tc.tile_pool(name=f"itE{it}", bufs=1) as itE:
                  # ---------- B1: windows + corr/lc; B2: transpose to spatial ----------
                  lc_all = pp.tile([128, 18, 75], F32, name="lc_all", tag="lc_all")
                  corrA = itE.tile([128, SLAB, WP], BF16, name="corrA", tag="corrA")
                  corrB = itE.tile([25, SLAB, WP], BF16, name="corrB", tag="corrB")
                  nc.vector.memset(corrA[:], 0.0)
                  nc.vector.memset(corrB[:], 0.0)
                  for t in range(NT):
                      eng = nc.vector if (t % 3) else nc.gpsimd
                      cpx = wk.tile([128, 160], BF16, name=f"cpx{t}", tag=f"cpx{t % 2}")
                      cpxv = cpx[:, 0:150].rearrange("p (k c) -> p k c", k=3)
                      off = wk.tile([128, 3], F32, name="off", tag=f"off{t % 2}")
                      nc.vector.tensor_scalar(off[:], dt_cur[:, t, :], 24.0, None, op0=OP.add)
                      nc.vector.tensor_scalar(off[:], off[:], 0.0, 96.0, op0=OP.max, op1=OP.min)
                      rem = off
                      cur = None
                      for bi, bit in enumerate(BITS):
                          wd = BWID[bit]
                          m = wk.tile([128, 3], F32, name="m", tag=f"m{t % 2}")
                          nc.vector.tensor_scalar(m[:], rem[:], float(bit), None, op0=OP.is_ge)
                          nc.vector.scalar_tensor_tensor(rem[:], m[:], float(-bit), rem[:],
                                                         op0=OP.mult, op1=OP.add)
                          if cur is None:
                              srcb = volp[:, t, bit:bit + wd].unsqueeze(1).broadcast_to([128, 3, wd])
                              src0 = volp[:, t, 0:wd].unsqueeze(1).broadcast_to([128, 3, wd])
                          else:
                              srcb = cur[:, bit:bit + wd] if len(cur.shape) == 2 else cur[:, :, bit:bit + wd]
                              src0 = cur[:, 0:wd] if len(cur.shape) == 2 else cur[:, :, 0:wd]
                          nxt = bp.tile([128, 3, BWID[64]], F32, name="nxt", tag="bs")
                          eng.tensor_tensor(nxt[:, :, 0:wd], srcb, src0, op=OP.subtract)
                          eng.tensor_tensor(nxt[:, :, 0:wd], nxt[:, :, 0:wd], m[:].unsqueeze(2).broadcast_to([128, 3, wd]), op=OP.mult)
                          eng.tensor_tensor(nxt[:, :, 0:wd], nxt[:, :, 0:wd], src0, op=OP.add)
                          cur = nxt[:, :, 0:wd]
                      g = cur  # AP view [128, 3, 52]
                      eng.tensor_copy(cpxv[:, :, 0:25], g[:, :, 13:38])
                      if 6 <= t < 24:
                          eng.tensor_copy(lc_all[:, t - 6, :].rearrange("p (k c) -> p k c", k=3),
                                          g[:, :, 13:38])
                      dti = wk.tile([128, 3], I32, name="dti", tag=f"dti{t % 2}")
                      nc.vector.tensor_copy(dti[:], dt_cur[:, t, :])
                      nc.vector.tensor_scalar(dti[:], dti[:], 1, None, op0=OP.bitwise_and)
                      par = wk.tile([128, 3], F32, name="par", tag=f"par{t % 2}")
                      nc.vector.tensor_copy(par[:], dti[:])
                      av = wk.tile([128, 3], F32, name="av", tag=f"av{t % 2}")
                      bv = wk.tile([128, 3], F32, name="bv", tag=f"bv{t % 2}")
                      nc.vector.tensor_scalar(av[:], par[:], 0.25, None, op0=OP.mult)
                      nc.vector.tensor_scalar(bv[:], par[:], -0.25, 0.5, op0=OP.mult, op1=OP.add)
                      t1 = wk.tile([128, 3, 25], F32, name="t1", tag=f"t1_{t % 2}")
                      t2 = wk.tile([128, 3, 25], F32, name="t2", tag=f"t2_{t % 2}")
                      eng.tensor_tensor(t1[:], g[:, :, 0:49:2], g[:, :, 3:52:2], op=OP.add)
                      eng.tensor_tensor(t2[:], g[:, :, 1:50:2], g[:, :, 2:51:2], op=OP.add)
                      eng.tensor_tensor(t1[:], t1[:], av[:].unsqueeze(2).broadcast_to([128, 3, 25]), op=OP.mult)
                      eng.tensor_tensor(t2[:], t2[:], bv[:].unsqueeze(2).broadcast_to([128, 3, 25]), op=OP.mult)
                      eng.tensor_tensor(cpxv[:, :, 25:50], t1[:], t2[:], op=OP.add)
                      eng.tensor_copy(cpx[:, 150:153], dt_cur[:, t, :])
                      # transpose this px tile to spatial layout
                      pa = pst.tile([128, 128], BF16, name="pa", tag="pstr")
                      pb = pst.tile([32, 128], BF16, name="pb", tag="pstr")
                      nc.tensor.transpose(pa[:], cpx[:, 0:128], ident[:])
                      nc.tensor.transpose(pb[:], cpx[:, 128:160], ident[:])
                      p0 = t * 128
                      while p0 < (t + 1) * 128:
                          r = p0 // W
                          run = min((t + 1) * 128, (r + 1) * W) - p0
                          w0 = p0 - r * W
                          c0 = p0 - t * 128
                          nc.scalar.copy(corrA[:, r, 1 + w0:1 + w0 + run], pa[:, c0:c0 + run])
                          nc.scalar.copy(corrB[:, r, 1 + w0:1 + w0 + run], pb[0:25, c0:c0 + run])
                          p0 += run

                  # ---------- B3: conv stack ----------
                  def conv_tiles(r_lo, r_hi):
                      r = r_lo
                      while r < r_hi:
                          nr = min(3, r_hi - r)
                          yield r, nr
                          r += nr

                  xbf = itE.tile([128, SLAB, WP], BF16, name="xbf", tag="xbf")
                  nc.vector.memset(xbf[:], 0.0)
                  for r0, nr in conv_tiles(1, 23):
                      ps = psc.tile([128, nr, W], F32, name="psE", tag="pscv")
                      for s in range(9):
                          dy, dx = s // 3, s % 3
                          nc.tensor.matmul(ps[:], wenc0[:, s, :], _conv_shift_rhs(corrA[:], r0, dy, dx, nr), start=(s == 0), stop=False)
                          nc.tensor.matmul(ps[:], wenc1[:, s, :], _conv_shift_rhs(corrB[:], r0, dy, dx, nr), start=False, stop=(s == 8))
                      nc.scalar.activation(xbf[:, r0:r0 + nr, 1:161], ps[:], AF.Relu, bias=benc[:])
                  nc.vector.tensor_tensor(xbf[:, 1:23, :], xbf[:, 1:23, :],
                                          mrow[:, 1:23].unsqueeze(2).broadcast_to([128, 22, WP]), op=OP.mult)

                  # r conv first (rows [2,22)) -> rnet
                  rnet = itE.tile([128, SLAB, WP], BF16, name="rnet", tag="rnet")
                  nc.vector.memset(rnet[:], 0.0)
                  for r0, nr in conv_tiles(2, 22):
                      ps = psc.tile([128, nr, W], F32, name="psr", tag="pscv")
                      for s in range(9):
                          dy, dx = s // 3, s % 3
                          nc.tensor.matmul(ps[:], wr0[:, s, :], _conv_shift_rhs(netbf[:], r0, dy, dx, nr), start=(s == 0), stop=False)
                          nc.tensor.matmul(ps[:], wr1[:, s, :], _conv_shift_rhs(xbf[:], r0, dy, dx, nr), start=False, stop=(s == 8))
                      cs = st.tile([128, 3, W], F32, name="csr", tag="cs")
                      nc.sync.dma_start(cs[:, 0:nr, :], CRS.ap()[:, r0 - 2:r0 - 2 + nr, :])
                      tadd = wk.tile([128, nr, W], F32, name="tar", tag="tadd")
                      nc.vector.tensor_tensor(tadd[:], ps[:], cs[:, 0:nr, :], op=OP.add)
                      rs = wk.tile([128, nr, W], F32, name="rs", tag="qs")
                      nc.scalar.activation(rs[:], tadd[:], AF.Sigmoid, bias=br[:])
                      nc.vector.tensor_tensor(rnet[:, r0:r0 + nr, 1:161], rs[:], net[:, r0:r0 + nr, 1:161], op=OP.mult)

                  # z conv + q conv + net update per 3-row block (rows [3,21))
                  for r0, nr in conv_tiles(3, 21):
                      psz = psc.tile([128, nr, W], F32, name="psz", tag="pscv")
                      for s in range(9):
                          dy, dx = s // 3, s % 3
                          nc.tensor.matmul(psz[:], wz0[:, s, :], _conv_shift_rhs(netbf[:], r0, dy, dx, nr), start=(s == 0), stop=False)
                          nc.tensor.matmul(psz[:], wz1[:, s, :], _conv_shift_rhs(xbf[:], r0, dy, dx, nr), start=False, stop=(s == 8))
                      csz = st.tile([128, 3, W], F32, name="csz", tag="cs")
                      nc.sync.dma_start(csz[:, 0:nr, :], CZS.ap()[:, r0 - 2:r0 - 2 + nr, :])
                      tz = wk.tile([128, nr, W], F32, name="tz", tag="tadd")
                      nc.vector.tensor_tensor(tz[:], psz[:], csz[:, 0:nr, :], op=OP.add)
                      zs = wk.tile([128, nr, W], F32, name="zs", tag="zs")
                      nc.scalar.activation(zs[:], tz[:], AF.Sigmoid, bias=bz[:])
                      ps = psc.tile([128, nr, W], F32, name="psq", tag="pscv")
                      for s in range(9):
                          dy, dx = s // 3, s % 3
                          nc.tensor.matmul(ps[:], wq0[:, s, :], _conv_shift_rhs(rnet[:], r0, dy, dx, nr), start=(s == 0), stop=False)
                          nc.tensor.matmul(ps[:], wq1[:, s, :], _conv_shift_rhs(xbf[:], r0, dy, dx, nr), start=False, stop=(s == 8))
                      cs = st.tile([128, 3, W], F32, name="csq", tag="cs")
                      nc.sync.dma_start(cs[:, 0:nr, :], CQS.ap()[:, r0 - 2:r0 - 2 + nr, :])
                      tadd = wk.tile([128, nr, W], F32, name="taq", tag="tadd")
                      nc.vector.tensor_tensor(tadd[:], ps[:], cs[:, 0:nr, :], op=OP.add)
                      qs = wk.tile([128, nr, W], F32, name="qs", tag="qs")
                      nc.scalar.activation(qs[:], tadd[:], AF.Tanh, bias=bq[:])
                      dqn = wk.tile([128, nr, W], F32, name="dqn", tag="dqn")
                      nc.vector.tensor_tensor(dqn[:], qs[:], net[:, r0:r0 + nr, 1:161], op=OP.subtract)
                      nc.vector.tensor_tensor(dqn[:], dqn[:], zs[:], op=OP.mult)
                      nc.vector.tensor_tensor(net[:, r0:r0 + nr, 1:161], net[:, r0:r0 + nr, 1:161], dqn[:], op=OP.add)
                  nc.vector.tensor_tensor(net[:, 3:21, :], net[:, 3:21, :],
                                          mrow[:, 3:21].unsqueeze(2).broadcast_to([128, 18, WP]), op=OP.mult)
                  nc.vector.tensor_copy(netbf[:, 3:21, :], net[:, 3:21, :])

              with tc.tile_pool(name=f"itL{it}", bufs=1) as itL:
                  # d1 (f32r), dlc=d2, me=m-conv
                  d1sl = itL.tile([128, 16, WP], F32R, name="d1sl", tag="d1sl")
                  nc.vector.memset(d1sl[:].bitcast(F32), 0.0)
                  for r0, nr in conv_tiles(4, 20):
                      ps = psc.tile([128, nr, W], F32, name="psd1", tag="pscv")
                      for s in range(9):
                          dy, dx = s // 3, s % 3
                          nc.tensor.matmul(ps[:], wd1[:, s, :], _conv_shift_rhs(net[:], r0, dy, dx, nr), start=(s == 0), stop=(s == 8))
                      nc.scalar.activation(d1sl[:, r0 - 4:r0 - 4 + nr, 1:161], ps[:], AF.Relu, bias=bd1[:])
                  nc.vector.tensor_tensor(d1sl[:], d1sl[:],
                                          mrow[:, 4:20].unsqueeze(2).broadcast_to([128, 16, WP]), op=OP.mult)

                  gsp = itL.tile([75, 2304], F32, name="gsp", tag="gsp")
                  mes0t = itL.tile([128, 2304], F32, name="mes0t", tag="mes0t")
                  mes1t = itL.tile([16, 2304], F32, name="mes1t", tag="mes1t")
                  dlcsp = gsp[:, :]
                  mesp1 = mes1t[:, :]
                  mesp0 = mes0t[:, :]
                  nc.vector.memset(gsp[:], 0.0)
                  for r0, nr in conv_tiles(5, 19):
                      col = r0 * W - 768
                      psf = psc.tile([128, nr, W], F32, name="psd2", tag="pscv")
                      ps = psf[0:75, :, :]
                      for s in range(9):
                          dy, dx = s // 3, s % 3
                          nc.tensor.matmul(ps, wd2[:, s, :], _conv_shift_rhs(d1sl[:], r0 - 4, dy, dx, nr), start=(s == 0), stop=(s == 8))
                      nc.scalar.activation(dlcsp[:, col:col + nr * W], ps.rearrange("p a b -> p (a b)"), AF.Identity, bias=bd2[:])
                      ps0 = psc.tile([128, nr, W], F32, name="psm0", tag="pscv")
                      ps1 = psc.tile([16, nr, W], F32, name="psm1", tag="pscv")
                      for s in range(9):
                          dy, dx = s // 3, s % 3
                          nc.tensor.matmul(ps0[:], wm0[:, s, :], _conv_shift_rhs(netbf[:], r0, dy, dx, nr), start=(s == 0), stop=(s == 8))
                      for s in range(9):
                          dy, dx = s // 3, s % 3
                          nc.tensor.matmul(ps1[:], wm1[:, s, :], _conv_shift_rhs(netbf[:], r0, dy, dx, nr), start=(s == 0), stop=(s == 8))
                      nc.scalar.activation(mesp0[:, col:col + nr * W], ps0[:].rearrange("p a b -> p (a b)"), AF.Exp, bias=bme0[:], scale=0.25)
                      nc.scalar.activation(mesp1[:, col:col + nr * W], ps1[:].rearrange("p a b -> p (a b)"), AF.Exp, bias=bme1[:], scale=0.25)

                  # ---------- B4: prob / disp / top-3 per px tile ----------
                  for t in range(6, 24):
                      col = (t - 6) * 128
                      pd = pst.tile([128, 75], F32, name="pd", tag="pstr")
                      nc.tensor.transpose(pd[:], dlcsp[:, col:col + 128], identf[0:75, 0:75])
                      e = wk.tile([128, 75], F32, name="e", tag="e")
                      nc.vector.tensor_tensor(e[:], pd[:], lc_all[:, t - 6, :], op=OP.add)
                      nc.scalar.activation(e[:], e[:], AF.Exp)
                      ssum = wk.tile([128, 1], F32, name="ssum", tag="ssum")
                      nc.vector.reduce_sum(ssum[:], e[:], axis=mybir.AxisListType.X)
                      srec = wk.tile([128, 1], F32, name="srec", tag="srec")
                      nc.vector.reciprocal(srec[:], ssum[:])
                      smp = wk.tile([128, 3, 25], F32, name="smp", tag="smp")
                      for k in range(3):
                          nc.vector.tensor_scalar(smp[:, k, :], deltas[:, :], dt_cur[:, t, k:k + 1], None, op0=OP.add)
                      es = wk.tile([128, 75], F32, name="es", tag="es")
                      nc.vector.tensor_tensor(es[:], e[:], smp[:].rearrange("p a b -> p (a b)"), op=OP.mult)
                      esum = wk.tile([128, 1], F32, name="esum", tag="esum")
                      nc.vector.reduce_sum(esum[:], es[:], axis=mybir.AxisListType.X)
                      disp = wk.tile([128, 1], F32, name="disp", tag="disp")
                      nc.vector.tensor_tensor(disp[:], esum[:], srec[:], op=OP.mult)
                      nc.vector.tensor_scalar(disp[:], disp[:], pxm[:, t:t + 1], 4.0, op0=OP.mult, op1=OP.mult)
                      nc.sync.dma_start(
                          bass.AP(tensor=dram_dispf[:].tensor, offset=t * 128, ap=[[1, 128], [1, 1]]),
                          disp[:])
                      # top-3 of e
                      tv = wk.tile([128, 8], F32, name="tv2", tag="tv2")
                      ti = wk.tile([128, 8], U32, name="ti2", tag="ti2")
                      nc.vector.max(tv[:], e[:])
                      nc.vector.max_index(ti[:], tv[:], e[:])
                      tif = wk.tile([128, 3], F32, name="tif2", tag="tif2")
                      nc.vector.tensor_copy(tif[:], ti[:, 0:3].bitcast(I32))
                      s1 = wk.tile([128, 3], F32, name="s1", tag="s1")
                      s2 = wk.tile([128, 3], F32, name="s2", tag="s2")
                      nc.vector.tensor_scalar(s1[:], tif[:], 25.0, None, op0=OP.is_ge)
                      nc.vector.tensor_scalar(s2[:], tif[:], 50.0, None, op0=OP.is_ge)
                      jv = wk.tile([128, 3], F32, name="jv", tag="jv")
                      nc.vector.scalar_tensor_tensor(jv[:], s1[:], -25.0, tif[:], op0=OP.mult, op1=OP.add)
                      nc.vector.scalar_tensor_tensor(jv[:], s2[:], -25.0, jv[:], op0=OP.mult, op1=OP.add)
                      d10 = wk.tile([128, 1], F32, name="d10", tag="d10")
                      d21 = wk.tile([128, 1], F32, name="d21", tag="d21")
                      nc.vector.tensor_tensor(d10[:], dt_cur[:, t, 1:2], dt_cur[:, t, 0:1], op=OP.subtract)
                      nc.vector.tensor_tensor(d21[:], dt_cur[:, t, 2:3], dt_cur[:, t, 1:2], op=OP.subtract)
                      v = wk.tile([128, 3], F32, name="v", tag="v")
                      nc.vector.tensor_scalar(v[:], s2[:], d21[:], dt_cur[:, t, 0:1], op0=OP.mult, op1=OP.add)
                      nc.vector.scalar_tensor_tensor(v[:], s1[:], d10[:], v[:], op0=OP.mult, op1=OP.add)
                      nc.vector.tensor_tensor(v[:], v[:], jv[:], op=OP.add)
                      nc.vector.tensor_scalar(dt_nxt[:, t, :], v[:], -12.0, None, op0=OP.add)

                  # ---------- B5: upsample own rows ----------
                  Dall = itL.tile([128, 16, 9], F32, name="Dall", tag="Dall")
                  for t in range(7, 23):
                      for jblk in range(3):
                          srcoff = t * 128 + (jblk - 1) * 160 - 1
                          nc.sync.dma_start(
                              Dall[:, t - 7, 3 * jblk:3 * jblk + 3],
                              bass.AP(tensor=dram_dispf[:].tensor, offset=srcoff, ap=[[1, 128], [1, 3]]))
                  for qq in (0, 3, 6):
                      nc.vector.tensor_tensor(Dall[:, :, qq], Dall[:, :, qq], w0m[:], op=OP.mult)
                  for qq in (2, 5, 8):
                      nc.vector.tensor_tensor(Dall[:, :, qq], Dall[:, :, qq], w159m[:], op=OP.mult)
                  for t in range(7, 23):
                      col = t * 128 - 768
                      pm0 = pst.tile([128, 128], F32, name="pm0", tag="pstr")
                      pm1 = pst.tile([128, 16], F32, name="pm1", tag="pstr")
                      nc.tensor.transpose(pm0[:], mesp0[:, col:col + 128], identf[:])
                      nc.tensor.transpose(pm1[:], mesp1[:, col:col + 128], identf[0:16, 0:16])
                      met = wk.tile([128, 144], F32, name="met", tag="met")
                      nc.scalar.copy(met[:, 0:128], pm0[:])
                      nc.scalar.copy(met[:, 128:144], pm1[:])
                      msum = wk.tile([128, 16], F32, name="msum", tag="msum")
                      nc.vector.reduce_sum(msum[:], met[:].rearrange("p (q f) -> p f q", q=9), axis=mybir.AxisListType.X)
                      mrec = wk.tile([128, 16], F32, name="mrec", tag="mrec")
                      nc.vector.reciprocal(mrec[:], msum[:])
                      acc = wk.tile([128, 16], F32, name="acc", tag="acc")
                      nc.vector.tensor_scalar(acc[:], met[:, 0:16], Dall[:, t - 7, 0:1], None, op0=OP.mult)
                      for qq in range(1, 9):
                          nc.vector.scalar_tensor_tensor(acc[:], met[:, 16 * qq:16 * qq + 16],
                                                         Dall[:, t - 7, qq:qq + 1], acc[:],
                                                         op0=OP.mult, op1=OP.add)
                      pred = wk.tile([128, 16], F32, name="pred", tag="pred")
                      nc.vector.tensor_tensor(pred[:], acc[:], mrec[:], op=OP.mult)
                      # output DMA per row-run
                      p0 = t * 128
                      while p0 < (t + 1) * 128:
                          r = p0 // W
                          run = min((t + 1) * 128, (r + 1) * W) - p0
                          w0 = p0 - r * W
                          if 6 <= r < 18:
                              dst = bass.AP(tensor=OUT.ap().tensor,
                                            offset=it * 48 * 640 + (r - 6) * 4 * 640 + w0 * 4,
                                            ap=[[4, run], [640, 4], [1, 4]])
                              nc.sync.dma_start(dst, pred[p0 - t * 128:p0 - t * 128 + run, :].rearrange("p (a b) -> p a b", a=4))
                          p0 += run

                  # ---------- B6: AllGather + halo assembly ----------
                  if it < ITERS - 1:
                      agin = agins[it]; agout = agouts[it]
                      nc.sync.dma_start(agin[:, 0:960].rearrange("p (a b) -> p a b", a=6), net[:, 6:12, 1:161].bitcast(F32))
                      nc.sync.dma_start(agin[:, 960:1920].rearrange("p (a b) -> p a b", a=6), net[:, 12:18, 1:161].bitcast(F32))
                      nc.sync.dma_start(agin[:, 1920:1968].rearrange("p (a b) -> p a b", a=16), dt_nxt[:, 7:23, :])
                      nc.gpsimd.collective_compute(
                          "AllGather", OP.bypass, replica_groups=[list(range(NC_))],
                          ins=[agin[:].opt()], outs=[agout[:].opt()])
                      vt = agp.tile([128, 960], F32, name="vt", tag="vt")
                      vb = agp.tile([128, 960], F32, name="vb", tag="vb")
                      nc.vector.memset(vt[:], 0.0)
                      nc.vector.memset(vb[:], 0.0)
                      dtv = []
                      for kk in range(16):
                          dvt = wk.tile([128, 3], F32, name=f"dtv{kk}", tag=f"dtv{kk}")
                          nc.vector.memset(dvt[:], 0.0)
                          dtv.append(dvt)
                      for rr in range(NC_):
                          stg = stp.tile([128, 1968], F32, name="stg", tag="stg")
                          nc.sync.dma_start(stg[:], agout[:][rr * 128:(rr + 1) * 128, :])
                          nc.vector.scalar_tensor_tensor(vt[:], stg[:, 960:1920], agnt[:, rr:rr + 1], vt[:], op0=OP.mult, op1=OP.add)
                          nc.vector.scalar_tensor_tensor(vb[:], stg[:, 0:960], agnb[:, rr:rr + 1], vb[:], op0=OP.mult, op1=OP.add)
                          for kk in range(16):
                              k = kk if kk < 8 else kk + 14
                              src_packed = (k + 15) if kk < 8 else (k - 15)
                              msrc = mdtt if kk < 8 else mdtb
                              mi = kk if kk < 8 else kk - 8
                              c0 = 1920 + 3 * (src_packed - 7)
                              nc.vector.scalar_tensor_tensor(dtv[kk][:], stg[:, c0:c0 + 3],
                                                             msrc[:, rr * 8 + mi:rr * 8 + mi + 1], dtv[kk][:], op0=OP.mult, op1=OP.add)
                      nc.vector.tensor_copy(net[:, 0:6, 1:161], vt[:].rearrange("p (a b) -> p a b", a=6))
                      nc.vector.tensor_copy(net[:, 18:24, 1:161], vb[:].rearrange("p (a b) -> p a b", a=6))
                      nc.vector.tensor_copy(netbf[:, 0:6, :], net[:, 0:6, :])
                      nc.vector.tensor_copy(netbf[:, 18:24, :], net[:, 18:24, :])
                      for kk in range(16):
                          k = kk if kk < 8 else kk + 14
                          mown = mdtot if kk < 8 else mdtob
                          mi = kk if kk < 8 else kk - 8
                          nc.vector.scalar_tensor_tensor(dt_nxt[:, k, :], dt_nxt[:, k, :],
                                                         mown[:, mi:mi + 1], dtv[kk][:], op0=OP.mult, op1=OP.add)
              if it == 0:
                  nc.sync.dma_start(DBGDT.ap(), dt_nxt[:])
                  nc.sync.dma_start(DBGNET.ap(), net[:].bitcast(F32))
              dt_cur, dt_nxt = dt_nxt, dt_cur

    nc.compile()
    return nc


def _prep_inputs(full):
    bf = ml_dtypes.bfloat16
    def lhsT(warr, kslice, mslice):
        # [out,in,3,3] -> [K, 9, M]
        w = warr[mslice, kslice]
        return np.ascontiguousarray(w.transpose(1, 2, 3, 0).reshape(w.shape[1], 9, w.shape[0]))

    enc_w = full['enc_w']; m_w = full['m_w']
    shared = {
        'WENC0': lhsT(enc_w, slice(0, 128), slice(None)).astype(bf),
        'WENC1': lhsT(enc_w, slice(128, 153), slice(None)).astype(bf),
        'WZ0': lhsT(full['gz_w'], slice(0, 128), slice(None)).astype(bf),
        'WZ1': lhsT(full['gz_w'], slice(128, 256), slice(None)).astype(bf),
        'WR0': lhsT(full['gr_w'], slice(0, 128), slice(None)).astype(bf),
        'WR1': lhsT(full['gr_w'], slice(128, 256), slice(None)).astype(bf),
        'WQ0': lhsT(full['gq_w'], slice(0, 128), slice(None)).astype(bf),
        'WQ1': lhsT(full['gq_w'], slice(128, 256), slice(None)).astype(bf),
        'WD1': lhsT(full['d1_w'], slice(None), slice(None)).astype(np.float32),
        'WD2': lhsT(full['d2_w'], slice(None), slice(None)).astype(np.float32),
        'WM0': lhsT(m_w, slice(None), slice(0, 128)).astype(bf),
        'WM1': lhsT(m_w, slice(None), slice(128, 144)).astype(bf),
        'BENC': full['enc_b'].reshape(128, 1).astype(np.float32),
        'BD1': full['d1_b'].reshape(128, 1).astype(np.float32),
        'BD2': full['d2_b'].reshape(75, 1).astype(np.float32),
        'BZ': full['gz_b'].reshape(128, 1).astype(np.float32),
        'BR': full['gr_b'].reshape(128, 1).astype(np.float32),
        'BQ': full['gq_b'].reshape(128, 1).astype(np.float32),
        'BME0': (0.25 * full['m_b'][0:128]).reshape(128, 1).astype(np.float32),
        'BME1': (0.25 * full['m_b'][128:144]).reshape(16, 1).astype(np.float32),
        'DELTAS': np.tile(np.arange(-RT, RT + 1, dtype=np.float32), (128, 1)),
    }
    in_maps = []
    for i in range(NC_):
        R0 = 12 * i - HALO
        rows = np.arange(R0, R0 + SLAB)
        inimg = ((rows >= 0) & (rows < H)).astype(np.float32)

        def slab(x, lo=0, hi=SLAB):
            out = np.zeros((x.shape[0], hi - lo, W), np.float32)
            for j in range(lo, hi):
                r = R0 + j
                if 0 <= r < H:
                    out[:, j - lo] = x[:, r]
            return out

        px_in = np.repeat(inimg, W)
        pxm = np.zeros((128, NT), np.float32)
        for t in range(NT):
            pxm[:, t] = px_in[t * 128:(t + 1) * 128]
        w0m = np.zeros((128, 16), np.float32); w159m = np.zeros((128, 16), np.float32)
        for t in range(7, 23):
            pxs = np.arange(t * 128, (t + 1) * 128)
            w0m[:, t - 7] = (pxs % W != 0).astype(np.float32)
            w159m[:, t - 7] = (pxs % W != W - 1).astype(np.float32)
        agnt = np.zeros((128, 8), np.float32); agnb = np.zeros((128, 8), np.float32)
        if i - 1 >= 0:
            agnt[:, i - 1] = 1.0
        if i + 1 < NC_:
            agnb[:, i + 1] = 1.0
        mdtt = np.zeros((128, 64), np.float32); mdtot = np.zeros((128, 8), np.float32)
        mdtb = np.zeros((128, 64), np.float32); mdtob = np.zeros((128, 8), np.float32)
        for kk in range(8):
            k = kk
            pxs = np.arange(k * 128, (k + 1) * 128)
            halo_part = (pxs < 6 * W).astype(np.float32) * px_in[pxs.clip(0, NPX - 1)]
            own_part = (pxs >= 6 * W).astype(np.float32)
            if i - 1 >= 0:
                mdtt[:, (i - 1) * 8 + kk] = halo_part
            mdtot[:, kk] = own_part
        for kk in range(8):
            k = kk + 22
            pxs = np.arange(k * 128, (k + 1) * 128)
            halo_part = (pxs >= 18 * W).astype(np.float32) * px_in[pxs.clip(0, NPX - 1)]
            own_part = (pxs < 18 * W).astype(np.float32)
            if i + 1 < NC_:
                mdtb[:, (i + 1) * 8 + kk] = halo_part
            mdtob[:, kk] = own_part
        m = dict(shared)
        m.update({
            'F1': slab(full['fmap1'][0]).reshape(2, 128, SLAB, W),
            'F2': slab(full['fmap2'][0]).reshape(2, 128, SLAB, W),
            'NET0': slab(full['net0'][0]),
            'CZS': slab(full['cz'][0], 2, 22), 'CQS': slab(full['cq'][0], 2, 22),
            'CRS': slab(full['cr'][0], 2, 22),
            'MROW': np.tile(inimg, (128, 1)),
            'PXM': pxm, 'W0M': w0m, 'W159M': w159m,
            'AGNT': agnt, 'AGNB': agnb,
            'MDTT': mdtt, 'MDTOT': mdtot, 'MDTB': mdtb, 'MDTOB': mdtob,
        })
        in_maps.append(m)
    return in_maps


def kernel(**inputs):
    assert int(inputs['iters']) == ITERS
    if 'nc' not in _cache:
        _cache['nc'] = build()
    full = {k: np.asarray(v) for k, v in inputs.items()}
    in_maps = _prep_inputs(full)
    res = bass_utils.run_bass_kernel_spmd(_cache['nc'], in_maps, core_ids=list(range(NC_)))
    global _last_res
    _last_res = res
    out = np.zeros((ITERS, 1, 1, 4 * H, 4 * W), np.float32)
    for i in range(NC_):
        out[:, 0, 0, 48 * i:48 * i + 48, :] = res.results[i]['OUT']
    return out



# revision 51
# speedup vs baseline: 1.7063x; 1.7063x over previous
# Trainium2 Bass kernel for MC-Stereo-like iterative disparity refinement.
# Self-contained: hardcodes shapes; shards H=96 across 8 NeuronCores (12 own
# rows + 6-row halo), refreshes halos between iterations with an AllGather.
import sys, os
sys.path.insert(0, '/opt/trn_rl_repo')
import numpy as np
import ml_dtypes

import concourse.bass as bass
import concourse.bacc as bacc
import concourse.mybir as mybir
import concourse.tile as tile
from concourse import bass_utils

F32 = mybir.dt.float32
F32R = mybir.dt.float32r
F16 = mybir.dt.float16
BF16 = mybir.dt.bfloat16
I32 = mybir.dt.int32
U32 = mybir.dt.uint32
U8 = mybir.dt.uint8
OP = mybir.AluOpType
AF = mybir.ActivationFunctionType

H, W, C, HID = 96, 160, 256, 128
K, RT, DMAX, FACTOR = 3, 12, 48, 4
S = 2 * RT + 1            # 25
NC_ = 8                   # cores
OWN, HALO, SLAB = 12, 6, 24
NPX = SLAB * W            # 3840
NT = NPX // 128           # 30 px tiles
WP = W + 2                # 162 padded width
VPAD, VW = 49, 184        # vol_pad: data at cols [49,97)
WIN = 52                  # gathered window width
# barrel stage widths (src width consumed per stage, high bit first)
BITS = [64, 32, 16, 8, 4, 2, 1]
BWID = {64: 115, 32: 83, 16: 67, 8: 59, 4: 55, 2: 53, 1: 52}
ITERS = 3
NK = NT * 3               # 90

_cache = {}
_last_res = None


def _conv_shift_rhs(slab_ap, r0, dy, dx, nrows):
    # rhs AP [K, nrows, 160] for conv output rows [r0, r0+nrows) at shift (dy,dx)
    return slab_ap[:, r0 + dy - 1:r0 + dy - 1 + nrows, dx:dx + W]


def build():
    nc = bacc.Bacc("TRN2", target_bir_lowering=False, debug=False,
                   num_devices=NC_)

    def inp(name, shape, dtype=F32):
        return nc.dram_tensor(name, list(shape), dtype, kind="ExternalInput")

    F1 = inp("F1", [2, 128, SLAB, W]); F2 = inp("F2", [2, 128, SLAB, W])
    NET0 = inp("NET0", [HID, SLAB, W])
    CZS = inp("CZS", [HID, 20, W]); CQS = inp("CQS", [HID, 20, W]); CRS = inp("CRS", [HID, 20, W])
    WENC0 = inp("WENC0", [128, 9, 128], BF16); WENC1 = inp("WENC1", [25, 9, 128], BF16)
    WZ0 = inp("WZ0", [128, 9, 128], BF16); WZ1 = inp("WZ1", [128, 9, 128], BF16)
    WR0 = inp("WR0", [128, 9, 128], BF16); WR1 = inp("WR1", [128, 9, 128], BF16)
    WQ0 = inp("WQ0", [128, 9, 128], BF16); WQ1 = inp("WQ1", [128, 9, 128], BF16)
    WD1 = inp("WD1", [128, 9, 128]); WD2 = inp("WD2", [128, 9, 75])
    WM0 = inp("WM0", [128, 9, 128], BF16); WM1 = inp("WM1", [128, 9, 16], BF16)
    BENC = inp("BENC", [128, 1]); BD1 = inp("BD1", [128, 1]); BD2 = inp("BD2", [75, 1])
    BZ = inp("BZ", [128, 1]); BR = inp("BR", [128, 1]); BQ = inp("BQ", [128, 1])
    BME0 = inp("BME0", [128, 1]); BME1 = inp("BME1", [16, 1])
    DELTAS = inp("DELTAS", [128, 25])
    MROW = inp("MROW", [128, SLAB])
    PXM = inp("PXM", [128, NT])
    W0M = inp("W0M", [128, 16]); W159M = inp("W159M", [128, 16])
    AGNT = inp("AGNT", [128, 8]); AGNB = inp("AGNB", [128, 8])
    MDTF = inp("MDTF", [128, NC_ * 48]); MDTOF = inp("MDTOF", [128, 48])

    OUT = nc.dram_tensor("OUT", [ITERS, 4 * OWN, 4 * W], F32, kind="ExternalOutput")

    with tile.TileContext(nc) as tc:
        with (
            tc.tile_pool(name="persist", bufs=1) as pp,
            tc.tile_pool(name="work", bufs=2) as wk,
            tc.tile_pool(name="stream", bufs=3) as st,
            tc.tile_pool(name="stgp", bufs=3) as stp,
            tc.tile_pool(name="agp", bufs=1) as agp,
            tc.tile_pool(name="psc", bufs=4, space="PSUM") as psc,
            tc.tile_pool(name="pst", bufs=4, space="PSUM") as pst,
            tc.tile_pool(name="dram", bufs=1, space="DRAM") as dr,
        ):
            # ---------- load persistent constants / weights ----------
            def load(t_dram, shape, dtype, name):
                t = pp.tile(list(shape), dtype, name=name, tag=name)
                if dtype in (BF16, F32R) and t_dram.dtype != dtype:
                    nc.gpsimd.dma_start(t[:], t_dram.ap())
                else:
                    nc.sync.dma_start(t[:], t_dram.ap())
                return t

            wenc0 = load(WENC0, (128, 9, 128), BF16, "wenc0")
            wenc1 = load(WENC1, (25, 9, 128), BF16, "wenc1")
            wz0 = load(WZ0, (128, 9, 128), BF16, "wz0"); wz1 = load(WZ1, (128, 9, 128), BF16, "wz1")
            wr0 = load(WR0, (128, 9, 128), BF16, "wr0"); wr1 = load(WR1, (128, 9, 128), BF16, "wr1")
            wq0 = load(WQ0, (128, 9, 128), BF16, "wq0"); wq1 = load(WQ1, (128, 9, 128), BF16, "wq1")
            wd1 = load(WD1, (128, 9, 128), BF16, "wd1"); wd2 = load(WD2, (128, 9, 75), BF16, "wd2")
            wm0 = load(WM0, (128, 9, 128), BF16, "wm0"); wm1 = load(WM1, (128, 9, 16), BF16, "wm1")
            benc = load(BENC, (128, 1), F32, "benc")
            bd1 = load(BD1, (128, 1), F32, "bd1"); bd2 = load(BD2, (75, 1), F32, "bd2")
            bz = load(BZ, (128, 1), F32, "bz"); br = load(BR, (128, 1), F32, "br"); bq = load(BQ, (128, 1), F32, "bq")
            bme0 = load(BME0, (128, 1), F32, "bme0"); bme1 = load(BME1, (16, 1), F32, "bme1")
            deltas = load(DELTAS, (128, 25), F32, "deltas")
            mrow = load(MROW, (128, SLAB), F32, "mrow")
            pxm = load(PXM, (128, NT), F32, "pxm")
            w0m = load(W0M, (128, 16), F32, "w0m"); w159m = load(W159M, (128, 16), F32, "w159m")
            agnt = load(AGNT, (128, 8), F32, "agnt"); agnb = load(AGNB, (128, 8), F32, "agnb")
            mdtf = load(MDTF, (128, NC_ * 48), F32, "mdtf")
            mdtof = load(MDTOF, (128, 48), F32, "mdtof")

            ident = pp.tile([128, 128], BF16, name="ident", tag="ident")
            ones128 = pp.tile([128, 128], BF16, name="ones128", tag="ones128")
            nc.vector.memset(ones128[:], 1.0)
            nc.gpsimd.affine_select(ident[:], ones128[:], pattern=[[-1, 128]], base=0,
                                    channel_multiplier=1, compare_op=OP.is_equal, fill=0.0)
            onesf = pp.tile([128, 128], F32, name="onesf", tag="onesf")
            nc.vector.memset(onesf[:], 1.0)
            identf = pp.tile([128, 128], F32, name="identf", tag="identf")
            nc.gpsimd.affine_select(identf[:], onesf[:], pattern=[[-1, 128]], base=0,
                                    channel_multiplier=1, compare_op=OP.is_equal, fill=0.0)

            # ---------- persistent state ----------
            volpb = pp.tile([128, NT, VW], BF16, name="volpb", tag="volpb")
            nc.vector.memset(volpb[:], 0.0)
            net = pp.tile([128, SLAB, WP], F32R, name="net", tag="net")
            netbf = pp.tile([128, SLAB, WP], BF16, name="netbf", tag="netbf")
            dtA = pp.tile([128, NT, 3], F32, name="dtA", tag="dtA")
            dtB = pp.tile([128, NT, 3], F32, name="dtB", tag="dtB")
            lc_all = pp.tile([128, 18, 75], BF16, name="lc_all", tag="lc_all")

            # ---------- Phase A: correlation volume ----------
            with tc.tile_pool(name="phA", bufs=1) as pA:
                volp = pA.tile([128, NT, 48], F32, name="volp", tag="volp")
                nc.vector.memset(volp[:], 0.0)
                Rg = dr.tile([SLAB, 160, 208], F32)  # reversed gram rows
                zk = wk.tile([128, 48], F32, name="zk", tag="zk")
                nc.vector.memset(zk[:], 0.0)
                for b in range(30):
                    nc.sync.dma_start(
                        bass.AP(tensor=Rg[:].tensor, offset=b * 128 * 208 + 160,
                                ap=[[208, 128], [1, 48]]), zk[:])
                for rh in range(2):
                    f1a = pA.tile([128, 2, 12, W], F32, name="f1a", tag="f1a")
                    f2a = pA.tile([128, 2, 12, W], F32, name="f2a", tag="f2a")
                    nc.sync.dma_start(f1a[:], F1.ap()[:, :, 12 * rh:12 * rh + 12, :].rearrange("c p r w -> p c r w"))
                    nc.sync.dma_start(f2a[:], F2.ap()[:, :, 12 * rh:12 * rh + 12, :].rearrange("c p r w -> p c r w"))
                    for r3 in range(4):
                        gs3 = pA.tile([128, 3, 2, W], F32, name="gs3", tag="gs3")
                        for j3 in range(3):
                            rr_ = r3 * 3 + j3
                            r = 12 * rh + rr_
                            for ci2, (w0, m) in enumerate(((0, 128), (128, 32))):
                                pg = pst.tile([128, W], F32, name="pg", tag="pstr")
                                for c in range(2):
                                    f2rc = f2a[:, c, rr_, :]
                                    rev = bass.AP(tensor=f2rc.tensor,
                                                  offset=f2rc.offset + 159,
                                                  ap=[list(f2rc.ap[0]), [-1, W]])
                                    nc.tensor.matmul(pg[:m, :], f1a[:, c, rr_, w0:w0 + m], rev,
                                                     start=(c == 0), stop=(c == 1))
                                nc.scalar.activation(gs3[:m, j3, ci2, :], pg[:m, :], AF.Copy, scale=1.0 / 16.0)
                        r0 = 12 * rh + r3 * 3
                        dst0 = bass.AP(tensor=Rg[:].tensor, offset=r0 * 33280,
                                       ap=[[208, 128], [33280, 3], [1, 160]])
                        nc.sync.dma_start(dst0, gs3[:, :, 0, :])
                        dst1 = bass.AP(tensor=Rg[:].tensor, offset=r0 * 33280 + 128 * 208,
                                       ap=[[208, 32], [33280, 3], [1, 160]])
                        nc.scalar.dma_start(dst1, gs3[0:32, :, 1, :])
                # diagonal extraction -> volp[:, t, VPAD:VPAD+48]
                rgf = Rg[:].flatten()
                for r in range(SLAB):
                    p0 = r * W
                    p1 = p0 + W
                    while p0 < p1:
                        t = p0 // 128
                        run = min(p1, (t + 1) * 128) - p0
                        w = p0 - r * W
                        src = bass.AP(tensor=rgf.tensor, offset=r * 33280 + w * 207 + 159,
                                      ap=[[207, run], [1, 48]])
                        deng = nc.sync if (p0 // 128) % 2 == 0 else nc.scalar
                        deng.dma_start(volp[p0 - 128 * t:p0 - 128 * t + run, t, :], src)
                        p0 += run
                # bf16 copy of the data region for the barrel (per-group, so it
                # pipelines with the diagonal-extraction DMAs)
                for g0 in range(0, NT, 6):
                    nc.gpsimd.tensor_copy(volpb[:, g0:g0 + 6, VPAD:VPAD + 48], volp[:, g0:g0 + 6, :])
                # initial top-3 (descending) of vol, masked by in-image
                for t in range(NT):
                    tv = wk.tile([128, 8], F32, name="tv", tag="tv")
                    ti = wk.tile([128, 8], U32, name="ti", tag="ti")
                    nc.vector.max(tv[:], volp[:, t, :])
                    nc.vector.max_index(ti[:], tv[:], volp[:, t, :])
                    tif = wk.tile([128, 3], F32, name="tif", tag="tif")
                    nc.vector.tensor_copy(tif[:], ti[:, 0:3].bitcast(I32))
                    nc.vector.tensor_scalar(dtA[:, t, :], tif[:], pxm[:, t:t + 1], None, op0=OP.mult)
            # net = tanh(net0); pads zero
            nc.vector.memset(net[:].bitcast(F32), 0.0)
            nc.gpsimd.dma_start(net[:, :, 1:161], NET0.ap())
            nc.scalar.activation(net[:, :, 1:161], net[:, :, 1:161], AF.Tanh)
            nc.vector.tensor_copy(netbf[:], net[:])

            dram_dispf = dr.tile([4224], F32)
            zk2 = wk.tile([128, 33], F32, name="zk2", tag="zk2")
            nc.vector.memset(zk2[:], 0.0)
            nc.sync.dma_start(
                bass.AP(tensor=dram_dispf[:].tensor, offset=0, ap=[[33, 128], [1, 33]]), zk2[:])
            agins, agouts, agdins, agdouts = [], [], [], []
            for _it in range(ITERS - 1):
                _ai = dr.tile([128, 1920], F32, name=f"agin{_it}", tag=f"agin{_it}")
                _ao = dr.tile([NC_ * 128, 1920], F32, addr_space="Shared", name=f"agout{_it}", tag=f"agout{_it}")
                _di = dr.tile([128, 48], F32, name=f"agdin{_it}", tag=f"agdin{_it}")
                _do = dr.tile([NC_ * 128, 48], F32, addr_space="Shared", name=f"agdout{_it}", tag=f"agdout{_it}")
                agins.append(_ai); agouts.append(_ao)
                agdins.append(_di); agdouts.append(_do)

            def conv_tiles(r_lo, r_hi):
                r = r_lo
                while r < r_hi:
                    nr = min(3, r_hi - r)
                    yield r, nr
                    r += nr

            def conv_layer(steps, r_lo, r_hi, drain, chunk=4, psum_p=128, border=None):
                # steps: list of (w_tile, src_tile, row_offset); each expands
                # to 9 shifts. s-outer: one weight load serves all blocks.
                blocks = list(conv_tiles(r_lo, r_hi))
                if border is not None:
                    blocks = [b for b in blocks if b[0] in border] + \
                             [b for b in blocks if b[0] not in border]
                flat = [(wt, s, src, roff) for (wt, src, roff) in steps for s in range(9)]
                nstep = len(flat)
                for i0 in range(0, len(blocks), chunk):
                    blk = blocks[i0:i0 + chunk]
                    pss = [psc.tile([psum_p, nr, W], F32, name=f"ps{r0}", tag="pscv")
                           for (r0, nr) in blk]
                    for si, (wt, s, src, roff) in enumerate(flat):
                        dy, dx = s // 3, s % 3
                        for bi2, (r0, nr) in enumerate(blk):
                            nc.tensor.matmul(pss[bi2][:], wt[:, s, :],
                                             _conv_shift_rhs(src, r0 + roff, dy, dx, nr),
                                             start=(si == 0), stop=(si == nstep - 1))
                    for bi2, (r0, nr) in enumerate(blk):
                        drain(pss[bi2], r0, nr)

            dt_cur, dt_nxt = dtA, dtB
            for it in range(ITERS):
              with tc.tile_pool(name=f"itE{it}", bufs=1) as itE:
                  corrA = itE.tile([128, SLAB, WP], BF16, name="corrA", tag="corrA")
                  corrB = itE.tile([25, SLAB, WP], BF16, name="corrB", tag="corrB")
                  cpx_all = itE.tile([128, NT, 160], BF16, name="cpx_all", tag="cpx_all")
                  nc.vector.memset(corrA[:], 0.0)
                  nc.vector.memset(corrB[:], 0.0)
                  nc.vector.memset(cpx_all[:, :, 153:160], 0.0)

                  # ---------- B1: barrel gather; masks in 3 tile-range batches ----------
                  # iteration 0: dt in [0,47] so off-24 in [0,47] -> 6 stages, base col 24
                  BITS_IT = BITS[1:] if it == 0 else BITS
                  BC = 24 if it == 0 else 0
                  IW = 115 if it == 0 else 179
                  mbs = itE.tile([128, 7, NK], U8, name="mbs", tag="mbs")
                  avb = itE.tile([128, NK], BF16, name="avb", tag="avb")
                  bvb = itE.tile([128, NK], BF16, name="bvb", tag="bvb")
                  for (lo_t, hi_t) in ((12, 18), (0, 12), (18, 30)):
                      nk = (hi_t - lo_t) * 3
                      c0 = lo_t * 3
                      dts = dt_cur[:, lo_t:hi_t, :].rearrange("p t k -> p (t k)")
                      offa = wk.tile([128, nk], F32, name="offa", tag="offa")
                      nc.vector.tensor_scalar(offa[:], dts, 24.0 - BC, None, op0=OP.add)
                      nc.vector.tensor_scalar(offa[:], offa[:], 0.0, 96.0, op0=OP.max, op1=OP.min)
                      rem = offa
                      for bi, bit in enumerate(BITS_IT):
                          mf = wk.tile([128, nk], F32, name="mf", tag=f"mf{bi % 2}")
                          nc.vector.tensor_scalar(mf[:], rem[:], float(bit), None, op0=OP.is_ge)
                          nc.vector.scalar_tensor_tensor(rem[:], mf[:], float(-bit), rem[:],
                                                         op0=OP.mult, op1=OP.add)
                          nc.scalar.copy(mbs[:, bi, c0:c0 + nk], mf[:])
                      dti = wk.tile([128, nk], I32, name="dti", tag="dti")
                      nc.vector.tensor_copy(dti[:], dts)
                      nc.vector.tensor_scalar(dti[:], dti[:], 1, None, op0=OP.bitwise_and)
                      parf = wk.tile([128, nk], F32, name="parf", tag="parf")
                      nc.vector.tensor_copy(parf[:], dti[:])
                      nc.vector.tensor_scalar(avb[:, c0:c0 + nk], parf[:], 0.25, None, op0=OP.mult)
                      nc.vector.tensor_scalar(bvb[:, c0:c0 + nk], parf[:], -0.25, 0.5, op0=OP.mult, op1=OP.add)
                      nc.vector.tensor_copy(cpx_all[:, lo_t:hi_t, 150:153], dt_cur[:, lo_t:hi_t, :])
                  TC = 6
                  with tc.tile_pool(name=f"bp{it}", bufs=1) as bpool:
                      bb = [bpool.tile([128, TC, 3, 179], BF16, name=f"bb{j}", tag=f"bb{j}")
                            for j in range(3)]
                      t1 = bpool.tile([128, TC, 3, 25], BF16, name="t1", tag="t1")
                      t2 = bpool.tile([128, TC, 3, 25], BF16, name="t2", tag="t2")
                      CHUNKS = [(12, 0), (0, 1), (6, 2), (18, 0), (24, 1)]
                      for (t0, bj) in CHUNKS:
                          tn = TC
                          buf = bb[bj]
                          nc.vector.tensor_copy(
                              buf[:, 0:tn, :, 0:IW],
                              volpb[:, t0:t0 + tn, BC:BC + IW].unsqueeze(2).broadcast_to([128, tn, 3, IW]))
                          for bi, bit in enumerate(BITS_IT):
                              wd = BWID[bit]
                              mbv = mbs[:, bi, 3 * t0:3 * (t0 + tn)] \
                                  .rearrange("p (t k) -> p t k", k=3).unsqueeze(3) \
                                  .broadcast_to([128, tn, 3, wd])
                              nc.vector.copy_predicated(buf[:, 0:tn, :, 0:wd], mbv,
                                                        buf[:, 0:tn, :, bit:bit + wd])
                          g = buf[:, 0:tn, :, 0:52]
                          for k in range(3):
                              nc.scalar.copy(cpx_all[:, t0:t0 + tn, 50 * k:50 * k + 25],
                                             g[:, :, k, 13:38])
                          lo = max(6, t0); hi = min(24, t0 + tn)
                          if lo < hi:
                              nc.scalar.copy(
                                  lc_all[:, lo - 6:hi - 6, :].rearrange("p t (k c) -> p t k c", k=3),
                                  g[:, lo - t0:hi - t0, :, 13:38])
                          nc.gpsimd.tensor_tensor(t1[:, 0:tn], g[:, :, :, 0:49:2], g[:, :, :, 3:52:2], op=OP.add)
                          nc.gpsimd.tensor_tensor(t2[:, 0:tn], g[:, :, :, 1:50:2], g[:, :, :, 2:51:2], op=OP.add)
                          nc.gpsimd.tensor_tensor(
                              t1[:, 0:tn], t1[:, 0:tn],
                              avb[:].rearrange("p (t k) -> p t k", k=3)[:, t0:t0 + tn, :]
                                  .unsqueeze(3).broadcast_to([128, tn, 3, 25]),
                              op=OP.mult)
                          nc.gpsimd.tensor_tensor(
                              t2[:, 0:tn], t2[:, 0:tn],
                              bvb[:].rearrange("p (t k) -> p t k", k=3)[:, t0:t0 + tn, :]
                                  .unsqueeze(3).broadcast_to([128, tn, 3, 25]),
                              op=OP.mult)
                          for k in range(3):
                              nc.gpsimd.tensor_tensor(cpx_all[:, t0:t0 + tn, 50 * k + 25:50 * k + 50],
                                                      t1[:, 0:tn, k, :], t2[:, 0:tn, k, :], op=OP.add)

                  # ---------- B2: transpose to spatial ----------
                  for t in range(NT):
                      pa = pst.tile([128, 128], BF16, name="pa", tag="pstr")
                      pb = pst.tile([32, 128], BF16, name="pb", tag="pstr")
                      nc.tensor.transpose(pa[:], cpx_all[:, t, 0:128], ident[:])
                      nc.tensor.transpose(pb[:], cpx_all[:, t, 128:160], ident[:])
                      p0 = t * 128
                      while p0 < (t + 1) * 128:
                          r = p0 // W
                          run = min((t + 1) * 128, (r + 1) * W) - p0
                          w0 = p0 - r * W
                          c0 = p0 - t * 128
                          nc.scalar.copy(corrA[:, r, 1 + w0:1 + w0 + run], pa[:, c0:c0 + run])
                          nc.scalar.copy(corrB[:, r, 1 + w0:1 + w0 + run], pb[0:25, c0:c0 + run])
                          p0 += run
                  # ---------- B3: conv stack (s-outer, weight-reload amortized) ----------
                  xbf = itE.tile([128, SLAB, WP], BF16, name="xbf", tag="xbf")
                  nc.vector.memset(xbf[:], 0.0)

                  def drain_enc(ps, r0, nr):
                      nc.scalar.activation(xbf[:, r0:r0 + nr, 1:161], ps[:], AF.Relu, bias=benc[:])
                  conv_layer([(wenc0, corrA[:], 0), (wenc1, corrB[:], 0)], 1, 23, drain_enc)
                  nc.vector.tensor_tensor(xbf[:, 1:23, :], xbf[:, 1:23, :],
                                          mrow[:, 1:23].unsqueeze(2).broadcast_to([128, 22, WP]), op=OP.mult)

                  # r conv first (rows [2,22)) -> rnet
                  rnet = itE.tile([128, SLAB, WP], BF16, name="rnet", tag="rnet")
                  nc.vector.memset(rnet[:], 0.0)

                  def drain_r(ps, r0, nr):
                      cs = st.tile([128, 3, W], F32, name="csr", tag="cs")
                      nc.sync.dma_start(cs[:, 0:nr, :], CRS.ap()[:, r0 - 2:r0 - 2 + nr, :])
                      tadd = wk.tile([128, nr, W], F32, name="tar", tag="tadd")
                      nc.vector.tensor_tensor(tadd[:], ps[:], cs[:, 0:nr, :], op=OP.add)
                      rs = wk.tile([128, nr, W], F32, name="rs", tag="qs")
                      nc.scalar.activation(rs[:], tadd[:], AF.Sigmoid, bias=br[:])
                      nc.vector.tensor_tensor(rnet[:, r0:r0 + nr, 1:161], rs[:], net[:, r0:r0 + nr, 1:161], op=OP.mult)
                  conv_layer([(wr0, netbf[:], 0), (wr1, xbf[:], 0)], 2, 22, drain_r)

                  # z conv (rows [3,21)) -> zs_all
                  zs_all = itE.tile([128, 18, W], BF16, name="zs_all", tag="zs_all")

                  def drain_z(ps, r0, nr):
                      cs = st.tile([128, 3, W], F32, name="csz", tag="cs")
                      nc.sync.dma_start(cs[:, 0:nr, :], CZS.ap()[:, r0 - 2:r0 - 2 + nr, :])
                      tz = wk.tile([128, nr, W], F32, name="tz", tag="tadd")
                      nc.vector.tensor_tensor(tz[:], ps[:], cs[:, 0:nr, :], op=OP.add)
                      nc.scalar.activation(zs_all[:, r0 - 3:r0 - 3 + nr, :], tz[:], AF.Sigmoid, bias=bz[:])
                  conv_layer([(wz0, netbf[:], 0), (wz1, xbf[:], 0)], 3, 21, drain_z)

                  # q conv + net update (rows [3,21))
                  def drain_q(ps, r0, nr):
                      cs = st.tile([128, 3, W], F32, name="csq", tag="cs")
                      nc.sync.dma_start(cs[:, 0:nr, :], CQS.ap()[:, r0 - 2:r0 - 2 + nr, :])
                      tadd = wk.tile([128, nr, W], F32, name="taq", tag="tadd")
                      nc.vector.tensor_tensor(tadd[:], ps[:], cs[:, 0:nr, :], op=OP.add)
                      qs = wk.tile([128, nr, W], F32, name="qs", tag="qs")
                      nc.scalar.activation(qs[:], tadd[:], AF.Tanh, bias=bq[:])
                      dqn = wk.tile([128, nr, W], F32, name="dqn", tag="dqn")
                      nc.vector.tensor_tensor(dqn[:], qs[:], net[:, r0:r0 + nr, 1:161], op=OP.subtract)
                      nc.vector.tensor_tensor(dqn[:], dqn[:], zs_all[:, r0 - 3:r0 - 3 + nr, :], op=OP.mult)
                      nc.vector.tensor_tensor(net[:, r0:r0 + nr, 1:161], net[:, r0:r0 + nr, 1:161], dqn[:], op=OP.add)
                  conv_layer([(wq0, rnet[:], 0), (wq1, xbf[:], 0)], 3, 21, drain_q)
                  nc.vector.tensor_tensor(net[:, 3:21, :], net[:, 3:21, :],
                                          mrow[:, 3:21].unsqueeze(2).broadcast_to([128, 18, WP]), op=OP.mult)
                  nc.vector.tensor_copy(netbf[:, 3:21, :], net[:, 3:21, :])
                  if it < ITERS - 1:
                      agin = agins[it]
                      nc.sync.dma_start(agin[:, 0:960].rearrange("p (a b) -> p a b", a=6), net[:, 6:12, 1:161].bitcast(F32))
                      nc.sync.dma_start(agin[:, 960:1920].rearrange("p (a b) -> p a b", a=6), net[:, 12:18, 1:161].bitcast(F32))
                      nc.gpsimd.collective_compute(
                          "AllGather", OP.bypass, replica_groups=[list(range(NC_))],
                          ins=[agin[:].opt()], outs=[agouts[it][:].opt()])

              with tc.tile_pool(name=f"itL{it}", bufs=1) as itL:
                  # d1 (bf16), dlc=d2, me=m-conv
                  d1sl = itL.tile([128, 16, WP], BF16, name="d1sl", tag="d1sl")
                  nc.vector.memset(d1sl[:], 0.0)

                  def drain_d1(ps, r0, nr):
                      nc.scalar.activation(d1sl[:, r0 - 4:r0 - 4 + nr, 1:161], ps[:], AF.Relu, bias=bd1[:])
                  conv_layer([(wd1, netbf[:], 0)], 4, 20, drain_d1)
                  nc.vector.tensor_tensor(d1sl[:], d1sl[:],
                                          mrow[:, 4:20].unsqueeze(2).broadcast_to([128, 16, WP]), op=OP.mult)

                  gsp = itL.tile([75, 2304], F32, name="gsp", tag="gsp")
                  mes0t = itL.tile([128, 2304], F32, name="mes0t", tag="mes0t")
                  mes1t = itL.tile([16, 2304], F32, name="mes1t", tag="mes1t")
                  dlcsp = gsp[:, :]
                  mesp1 = mes1t[:, :]
                  mesp0 = mes0t[:, :]
                  nc.vector.memset(gsp[:], 0.0)

                  def drain_d2(ps, r0, nr):
                      col = r0 * W - 768
                      nc.scalar.activation(dlcsp[:, col:col + nr * W],
                                           ps[:].rearrange("p a b -> p (a b)"),
                                           AF.Identity, bias=bd2[:])

                  def drain_m0(ps, r0, nr):
                      col = r0 * W - 768
                      nc.scalar.activation(mesp0[:, col:col + nr * W],
                                           ps[:].rearrange("p a b -> p (a b)"),
                                           AF.Exp, bias=bme0[:], scale=0.25)

                  def drain_m1(ps, r0, nr):
                      col = r0 * W - 768
                      nc.scalar.activation(mesp1[:, col:col + nr * W],
                                           ps[:].rearrange("p a b -> p (a b)"),
                                           AF.Exp, bias=bme1[:], scale=0.25)
                  conv_layer([(wd2, d1sl[:], -4)], 5, 19, drain_d2, psum_p=75)


                  # ---------- B4: prob / disp / top-3, groups of 6 tiles ----------
                  for grp in range(3):
                      t0 = 6 + grp * 6
                      pdg = pst.tile([128, 6, 75], F32, name="pdg", tag="pstr")
                      for j in range(6):
                          col = (t0 - 6 + j) * 128
                          nc.tensor.transpose(pdg[:, j, :], dlcsp[:, col:col + 128], identf[0:75, 0:75])
                      eg = wk.tile([128, 6, 75], F32, name="eg", tag="eg")
                      nc.vector.tensor_tensor(eg[:], pdg[:], lc_all[:, t0 - 6:t0, :], op=OP.add)
                      nc.scalar.activation(eg[:], eg[:], AF.Exp)
                      Ek = wk.tile([128, 6, 3], F32, name="Ek", tag="Ek")
                      nc.vector.reduce_sum(Ek[:], eg[:].rearrange("p t (k c) -> p t k c", k=3),
                                           axis=mybir.AxisListType.X)
                      ssum = wk.tile([128, 6], F32, name="ssum", tag="ssum")
                      nc.vector.reduce_sum(ssum[:], Ek[:], axis=mybir.AxisListType.X)
                      srec = wk.tile([128, 6], F32, name="srec", tag="srec")
                      nc.vector.reciprocal(srec[:], ssum[:])
                      es = wk.tile([128, 6, 75], F32, name="es", tag="es")
                      nc.vector.tensor_tensor(
                          es[:].rearrange("p t (k c) -> p t k c", k=3),
                          eg[:].rearrange("p t (k c) -> p t k c", k=3),
                          deltas[:].unsqueeze(1).unsqueeze(1).broadcast_to([128, 6, 3, 25]), op=OP.mult)
                      esum = wk.tile([128, 6], F32, name="esum", tag="esum")
                      nc.vector.reduce_sum(esum[:], es[:], axis=mybir.AxisListType.X)
                      dtE = wk.tile([128, 6, 3], F32, name="dtE", tag="dtE")
                      nc.vector.tensor_tensor(dtE[:], Ek[:], dt_cur[:, t0:t0 + 6, :], op=OP.mult)
                      desum = wk.tile([128, 6], F32, name="desum", tag="desum")
                      nc.vector.reduce_sum(desum[:], dtE[:], axis=mybir.AxisListType.X)
                      disp = wk.tile([128, 6], F32, name="disp", tag="disp")
                      nc.vector.tensor_tensor(disp[:], esum[:], desum[:], op=OP.add)
                      nc.vector.tensor_tensor(disp[:], disp[:], srec[:], op=OP.mult)
                      nc.vector.tensor_tensor(disp[:], disp[:], pxm[:, t0:t0 + 6], op=OP.mult)
                      nc.vector.tensor_scalar(disp[:], disp[:], 4.0, None, op0=OP.mult)
                      nc.sync.dma_start(
                          bass.AP(tensor=dram_dispf[:].tensor, offset=t0 * 128, ap=[[1, 128], [128, 6]]),
                          disp[:])
                      if it < ITERS - 1:
                          tif_g = wk.tile([128, 6, 3], F32, name="tifg", tag="tifg")
                          for j in range(6):
                              tv = wk.tile([128, 8], F32, name="tv2", tag="tv2")
                              ti = wk.tile([128, 8], U32, name="ti2", tag="ti2")
                              nc.vector.max(tv[:], eg[:, j, :])
                              nc.vector.max_index(ti[:], tv[:], eg[:, j, :])
                              nc.vector.tensor_copy(tif_g[:, j, :], ti[:, 0:3].bitcast(I32))
                          s1 = wk.tile([128, 6, 3], F32, name="s1", tag="s1")
                          s2 = wk.tile([128, 6, 3], F32, name="s2", tag="s2")
                          nc.vector.tensor_scalar(s1[:], tif_g[:], 25.0, None, op0=OP.is_ge)
                          nc.vector.tensor_scalar(s2[:], tif_g[:], 50.0, None, op0=OP.is_ge)
                          jv = wk.tile([128, 6, 3], F32, name="jv", tag="jv")
                          nc.vector.scalar_tensor_tensor(jv[:], s1[:], -25.0, tif_g[:], op0=OP.mult, op1=OP.add)
                          nc.vector.scalar_tensor_tensor(jv[:], s2[:], -25.0, jv[:], op0=OP.mult, op1=OP.add)
                          d10 = wk.tile([128, 6, 1], F32, name="d10", tag="d10")
                          d21 = wk.tile([128, 6, 1], F32, name="d21", tag="d21")
                          nc.vector.tensor_tensor(d10[:], dt_cur[:, t0:t0 + 6, 1:2], dt_cur[:, t0:t0 + 6, 0:1], op=OP.subtract)
                          nc.vector.tensor_tensor(d21[:], dt_cur[:, t0:t0 + 6, 2:3], dt_cur[:, t0:t0 + 6, 1:2], op=OP.subtract)
                          v = wk.tile([128, 6, 3], F32, name="v", tag="v")
                          nc.vector.tensor_tensor(v[:], s2[:], d21[:].broadcast_to([128, 6, 3]), op=OP.mult)
                          nc.vector.tensor_tensor(v[:], v[:], dt_cur[:, t0:t0 + 6, 0:1].broadcast_to([128, 6, 3]), op=OP.add)
                          tmpv = wk.tile([128, 6, 3], F32, name="tmpv", tag="tmpv")
                          nc.vector.tensor_tensor(tmpv[:], s1[:], d10[:].broadcast_to([128, 6, 3]), op=OP.mult)
                          nc.vector.tensor_tensor(v[:], v[:], tmpv[:], op=OP.add)
                          nc.vector.tensor_tensor(v[:], v[:], jv[:], op=OP.add)
                          nc.vector.tensor_scalar(dt_nxt[:, t0:t0 + 6, :], v[:], -12.0, None, op0=OP.add)

                  conv_layer([(wm0, netbf[:], 0)], 5, 19, drain_m0)
                  conv_layer([(wm1, netbf[:], 0)], 5, 19, drain_m1, psum_p=16)

                  # ---------- B6a: dt AllGather + net stage loads ----------
                  stgs = []
                  if it < ITERS - 1:
                      agdin = agdins[it]
                      nc.sync.dma_start(agdin[:, 0:48].rearrange("p (a b) -> p a b", a=16), dt_nxt[:, 7:23, :])
                      nc.gpsimd.collective_compute(
                          "AllGather", OP.bypass, replica_groups=[list(range(NC_))],
                          ins=[agdin[:].opt()], outs=[agdouts[it][:].opt()])
                      for rr in range(NC_):
                          stg = stp.tile([128, 1920], BF16, name=f"stg{rr}", tag="stg")
                          nc.gpsimd.dma_start(stg[:], agouts[it][:][rr * 128:(rr + 1) * 128, :])
                          stgs.append(stg)

                  # ---------- B5: upsample own rows ----------
                  Dall = itL.tile([128, 16, 9], F32, name="Dall", tag="Dall")
                  for jblk in range(3):
                      srcoff = 7 * 128 + (jblk - 1) * 160 - 1
                      nc.scalar.dma_start(
                          Dall[:, :, 3 * jblk:3 * jblk + 3],
                          bass.AP(tensor=dram_dispf[:].tensor, offset=srcoff,
                                  ap=[[1, 128], [128, 16], [1, 3]]))
                  for qq in (0, 3, 6):
                      nc.vector.tensor_tensor(Dall[:, :, qq], Dall[:, :, qq], w0m[:], op=OP.mult)
                  for qq in (2, 5, 8):
                      nc.vector.tensor_tensor(Dall[:, :, qq], Dall[:, :, qq], w159m[:], op=OP.mult)
                  met_all = itL.tile([128, 16, 144], F32, name="met_all", tag="met_all")
                  for t in range(7, 23):
                      col = t * 128 - 768
                      pm0 = pst.tile([128, 128], F32, name="pm0", tag="pstr")
                      pm1 = pst.tile([128, 16], F32, name="pm1", tag="pstr")
                      nc.tensor.transpose(pm0[:], mesp0[:, col:col + 128], identf[:])
                      nc.tensor.transpose(pm1[:], mesp1[:, col:col + 128], identf[0:16, 0:16])
                      nc.scalar.copy(met_all[:, t - 7, 0:128], pm0[:])
                      nc.scalar.copy(met_all[:, t - 7, 128:144], pm1[:])
                  msum = wk.tile([128, 16, 16], F32, name="msum", tag="msum")
                  nc.vector.reduce_sum(msum[:], met_all[:].rearrange("p t (q f) -> p t f q", q=9),
                                       axis=mybir.AxisListType.X)
                  mrec = wk.tile([128, 16, 16], F32, name="mrec", tag="mrec")
                  nc.vector.reciprocal(mrec[:], msum[:])
                  acc = wk.tile([128, 16, 16], F32, name="acc", tag="acc")
                  tmpm = wk.tile([128, 16, 16], F32, name="tmpm", tag="tmpm")
                  nc.vector.tensor_tensor(
                      acc[:], met_all[:, :, 0:16],
                      Dall[:, :, 0].unsqueeze(2).broadcast_to([128, 16, 16]), op=OP.mult)
                  for qq in range(1, 9):
                      nc.vector.tensor_tensor(
                          tmpm[:], met_all[:, :, 16 * qq:16 * qq + 16],
                          Dall[:, :, qq].unsqueeze(2).broadcast_to([128, 16, 16]), op=OP.mult)
                      nc.vector.tensor_tensor(acc[:], acc[:], tmpm[:], op=OP.add)
                  pred = wk.tile([128, 16, 16], F32, name="pred", tag="pred")
                  nc.vector.tensor_tensor(pred[:], acc[:], mrec[:], op=OP.mult)
                  for t in range(7, 23):
                      p0 = t * 128
                      while p0 < (t + 1) * 128:
                          r = p0 // W
                          run = min((t + 1) * 128, (r + 1) * W) - p0
                          w0 = p0 - r * W
                          if 6 <= r < 18:
                              dst = bass.AP(tensor=OUT.ap().tensor,
                                            offset=it * 48 * 640 + (r - 6) * 4 * 640 + w0 * 4,
                                            ap=[[4, run], [640, 4], [1, 4]])
                              nc.sync.dma_start(dst, pred[p0 - t * 128:p0 - t * 128 + run, t - 7, :].rearrange("p (a b) -> p a b", a=4))
                          p0 += run

                  # ---------- B6b: consume AllGathers ----------
                  if it < ITERS - 1:
                      vt = agp.tile([128, 960], F32, name="vt", tag="vt")
                      vb = agp.tile([128, 960], F32, name="vb", tag="vb")
                      dtv = agp.tile([128, 48], F32, name="dtv", tag="dtv")
                      nc.vector.memset(vt[:], 0.0)
                      nc.vector.memset(vb[:], 0.0)
                      nc.vector.memset(dtv[:], 0.0)
                      dttmp = wk.tile([128, 48], F32, name="dttmp", tag="dttmp")
                      stgd_all = stp.tile([128, NC_, 48], F32, name="stgd_all", tag="stgd_all")
                      nc.sync.dma_start(
                          stgd_all[:],
                          bass.AP(tensor=agdouts[it][:].tensor, offset=0,
                                  ap=[[48, 128], [128 * 48, NC_], [1, 48]]))
                      for rr in range(NC_):
                          stg = stgs[rr]
                          nc.vector.scalar_tensor_tensor(vt[:], stg[:, 960:1920], agnt[:, rr:rr + 1], vt[:], op0=OP.mult, op1=OP.add)
                          nc.vector.scalar_tensor_tensor(vb[:], stg[:, 0:960], agnb[:, rr:rr + 1], vb[:], op0=OP.mult, op1=OP.add)
                          nc.vector.tensor_tensor(dttmp[:], stgd_all[:, rr, :],
                                                  mdtf[:, rr * 48:(rr + 1) * 48], op=OP.mult)
                          nc.vector.tensor_tensor(dtv[:], dtv[:], dttmp[:], op=OP.add)
                      nc.vector.tensor_copy(net[:, 0:6, 1:161], vt[:].rearrange("p (a b) -> p a b", a=6))
                      nc.vector.tensor_copy(net[:, 18:24, 1:161], vb[:].rearrange("p (a b) -> p a b", a=6))
                      nc.vector.tensor_copy(netbf[:, 0:6, :], net[:, 0:6, :])
                      nc.vector.tensor_copy(netbf[:, 18:24, :], net[:, 18:24, :])
                      topv = dt_nxt[:, 0:8, :].rearrange("p t k -> p (t k)")
                      botv = dt_nxt[:, 22:30, :].rearrange("p t k -> p (t k)")
                      nc.vector.tensor_tensor(topv, topv, mdtof[:, 24:48], op=OP.mult)
                      nc.vector.tensor_tensor(topv, topv, dtv[:, 24:48], op=OP.add)
                      nc.vector.tensor_tensor(botv, botv, mdtof[:, 0:24], op=OP.mult)
                      nc.vector.tensor_tensor(botv, botv, dtv[:, 0:24], op=OP.add)
              dt_cur, dt_nxt = dt_nxt, dt_cur

    nc.compile()
    return nc


def _prep_inputs(full):
    bf = ml_dtypes.bfloat16
    def lhsT(warr, kslice, mslice):
        # [out,in,3,3] -> [K, 9, M]
        w = warr[mslice, kslice]
        return np.ascontiguousarray(w.transpose(1, 2, 3, 0).reshape(w.shape[1], 9, w.shape[0]))

    enc_w = full['enc_w']; m_w = full['m_w']
    shared = {
        'WENC0': lhsT(enc_w, slice(0, 128), slice(None)).astype(bf),
        'WENC1': lhsT(enc_w, slice(128, 153), slice(None)).astype(bf),
        'WZ0': lhsT(full['gz_w'], slice(0, 128), slice(None)).astype(bf),
        'WZ1': lhsT(full['gz_w'], slice(128, 256), slice(None)).astype(bf),
        'WR0': lhsT(full['gr_w'], slice(0, 128), slice(None)).astype(bf),
        'WR1': lhsT(full['gr_w'], slice(128, 256), slice(None)).astype(bf),
        'WQ0': lhsT(full['gq_w'], slice(0, 128), slice(None)).astype(bf),
        'WQ1': lhsT(full['gq_w'], slice(128, 256), slice(None)).astype(bf),
        'WD1': lhsT(full['d1_w'], slice(None), slice(None)).astype(np.float32),
        'WD2': lhsT(full['d2_w'], slice(None), slice(None)).astype(np.float32),
        'WM0': lhsT(m_w, slice(None), slice(0, 128)).astype(bf),
        'WM1': lhsT(m_w, slice(None), slice(128, 144)).astype(bf),
        'BENC': full['enc_b'].reshape(128, 1).astype(np.float32),
        'BD1': full['d1_b'].reshape(128, 1).astype(np.float32),
        'BD2': full['d2_b'].reshape(75, 1).astype(np.float32),
        'BZ': full['gz_b'].reshape(128, 1).astype(np.float32),
        'BR': full['gr_b'].reshape(128, 1).astype(np.float32),
        'BQ': full['gq_b'].reshape(128, 1).astype(np.float32),
        'BME0': (0.25 * full['m_b'][0:128]).reshape(128, 1).astype(np.float32),
        'BME1': (0.25 * full['m_b'][128:144]).reshape(16, 1).astype(np.float32),
        'DELTAS': np.tile(np.arange(-RT, RT + 1, dtype=np.float32), (128, 1)),
    }
    in_maps = []
    for i in range(NC_):
        R0 = 12 * i - HALO
        rows = np.arange(R0, R0 + SLAB)
        inimg = ((rows >= 0) & (rows < H)).astype(np.float32)

        def slab(x, lo=0, hi=SLAB):
            out = np.zeros((x.shape[0], hi - lo, W), np.float32)
            for j in range(lo, hi):
                r = R0 + j
                if 0 <= r < H:
                    out[:, j - lo] = x[:, r]
            return out

        px_in = np.repeat(inimg, W)
        pxm = np.zeros((128, NT), np.float32)
        for t in range(NT):
            pxm[:, t] = px_in[t * 128:(t + 1) * 128]
        w0m = np.zeros((128, 16), np.float32); w159m = np.zeros((128, 16), np.float32)
        for t in range(7, 23):
            pxs = np.arange(t * 128, (t + 1) * 128)
            w0m[:, t - 7] = (pxs % W != 0).astype(np.float32)
            w159m[:, t - 7] = (pxs % W != W - 1).astype(np.float32)
        agnt = np.zeros((128, 8), np.float32); agnb = np.zeros((128, 8), np.float32)
        if i - 1 >= 0:
            agnt[:, i - 1] = 1.0
        if i + 1 < NC_:
            agnb[:, i + 1] = 1.0
        # batched dt-halo masks: dtv col j (0..47) maps to sender packed tile
        # sp = 7 + j//3; bottom tiles (22..29) come from rank i+1 (sp 7..14),
        # top tiles (0..7) from rank i-1 (sp 15..22).
        mdtf = np.zeros((128, NC_ * 48), np.float32)
        mdtof = np.zeros((128, 48), np.float32)
        for kk in range(8):  # top tiles k = kk
            pxs = np.arange(kk * 128, (kk + 1) * 128)
            halo = (pxs < 6 * W).astype(np.float32) * px_in[pxs.clip(0, NPX - 1)]
            own = (pxs >= 6 * W).astype(np.float32)
            if i - 1 >= 0:
                for c in range(3):
                    mdtf[:, (i - 1) * 48 + 24 + 3 * kk + c] = halo
            for c in range(3):
                mdtof[:, 24 + 3 * kk + c] = own
        for kk in range(8):  # bottom tiles k = kk + 22
            k = kk + 22
            pxs = np.arange(k * 128, (k + 1) * 128)
            halo = (pxs >= 18 * W).astype(np.float32) * px_in[pxs.clip(0, NPX - 1)]
            own = (pxs < 18 * W).astype(np.float32)
            if i + 1 < NC_:
                for c in range(3):
                    mdtf[:, (i + 1) * 48 + 3 * kk + c] = halo
            for c in range(3):
                mdtof[:, 3 * kk + c] = own
        m = dict(shared)
        m.update({
            'F1': slab(full['fmap1'][0]).reshape(2, 128, SLAB, W),
            'F2': slab(full['fmap2'][0]).reshape(2, 128, SLAB, W),
            'NET0': slab(full['net0'][0]),
            'CZS': slab(full['cz'][0], 2, 22), 'CQS': slab(full['cq'][0], 2, 22),
            'CRS': slab(full['cr'][0], 2, 22),
            'MROW': np.tile(inimg, (128, 1)),
            'PXM': pxm, 'W0M': w0m, 'W159M': w159m,
            'AGNT': agnt, 'AGNB': agnb,
            'MDTF': mdtf, 'MDTOF': mdtof,
        })
        in_maps.append(m)
    return in_maps


def kernel(**inputs):
    assert int(inputs['iters']) == ITERS
    if 'nc' not in _cache:
        _cache['nc'] = build()
    full = {k: np.asarray(v) for k, v in inputs.items()}
    in_maps = _prep_inputs(full)
    res = bass_utils.run_bass_kernel_spmd(_cache['nc'], in_maps, core_ids=list(range(NC_)))
    global _last_res
    _last_res = res
    out = np.zeros((ITERS, 1, 1, 4 * H, 4 * W), np.float32)
    for i in range(NC_):
        out[:, 0, 0, 48 * i:48 * i + 48, :] = res.results[i]['OUT']
    return out


# revision 54
# speedup vs baseline: 1.8553x; 1.0873x over previous
# Trainium2 Bass kernel for MC-Stereo-like iterative disparity refinement.
# Self-contained: hardcodes shapes; shards H=96 across 8 NeuronCores (12 own
# rows + 6-row halo), refreshes halos between iterations with an AllGather.
import sys, os
sys.path.insert(0, '/opt/trn_rl_repo')
import numpy as np
import ml_dtypes

import concourse.bass as bass
import concourse.bacc as bacc
import concourse.mybir as mybir
import concourse.tile as tile
from concourse import bass_utils

F32 = mybir.dt.float32
F32R = mybir.dt.float32r
F16 = mybir.dt.float16
BF16 = mybir.dt.bfloat16
I32 = mybir.dt.int32
U32 = mybir.dt.uint32
U8 = mybir.dt.uint8
OP = mybir.AluOpType
AF = mybir.ActivationFunctionType

H, W, C, HID = 96, 160, 256, 128
K, RT, DMAX, FACTOR = 3, 12, 48, 4
S = 2 * RT + 1            # 25
NC_ = 8                   # cores
OWN, HALO, SLAB = 12, 6, 24
NPX = SLAB * W            # 3840
NT = NPX // 128           # 30 px tiles
WP = W + 2                # 162 padded width
VPAD, VW = 49, 184        # vol_pad: data at cols [49,97)
WIN = 52                  # gathered window width
# barrel stage widths (src width consumed per stage, high bit first)
BITS = [64, 32, 16, 8, 4, 2, 1]
BWID = {64: 115, 32: 83, 16: 67, 8: 59, 4: 55, 2: 53, 1: 52}
ITERS = 3
NK = NT * 3               # 90

_cache = {}
_last_res = None


def _conv_shift_rhs(slab_ap, r0, dy, dx, nrows):
    # rhs AP [K, nrows, 160] for conv output rows [r0, r0+nrows) at shift (dy,dx)
    return slab_ap[:, r0 + dy - 1:r0 + dy - 1 + nrows, dx:dx + W]


def build():
    nc = bacc.Bacc("TRN2", target_bir_lowering=False, debug=False,
                   num_devices=NC_)

    def inp(name, shape, dtype=F32):
        return nc.dram_tensor(name, list(shape), dtype, kind="ExternalInput")

    F1 = inp("F1", [2, 128, SLAB, W]); F2 = inp("F2", [2, 128, SLAB, W])
    NET0 = inp("NET0", [HID, SLAB, W])
    CZS = inp("CZS", [HID, 20, W]); CQS = inp("CQS", [HID, 20, W]); CRS = inp("CRS", [HID, 20, W])
    WENC0 = inp("WENC0", [128, 9, 128], BF16); WENC1 = inp("WENC1", [25, 9, 128], BF16)
    WZ0 = inp("WZ0", [128, 9, 128], BF16); WZ1 = inp("WZ1", [128, 9, 128], BF16)
    WR0 = inp("WR0", [128, 9, 128], BF16); WR1 = inp("WR1", [128, 9, 128], BF16)
    WQ0 = inp("WQ0", [128, 9, 128], BF16); WQ1 = inp("WQ1", [128, 9, 128], BF16)
    WD1 = inp("WD1", [128, 9, 128]); WD2 = inp("WD2", [128, 9, 75])
    WM0 = inp("WM0", [128, 9, 128], BF16); WM1 = inp("WM1", [128, 9, 16], BF16)
    BENC = inp("BENC", [128, 1]); BD1 = inp("BD1", [128, 1]); BD2 = inp("BD2", [75, 1])
    BZ = inp("BZ", [128, 1]); BR = inp("BR", [128, 1]); BQ = inp("BQ", [128, 1])
    BME0 = inp("BME0", [128, 1]); BME1 = inp("BME1", [16, 1])
    DELTAS = inp("DELTAS", [128, 25])
    MROW = inp("MROW", [128, SLAB])
    PXM = inp("PXM", [128, NT])
    W0M = inp("W0M", [128, 16]); W159M = inp("W159M", [128, 16])
    AGNT = inp("AGNT", [128, 8]); AGNB = inp("AGNB", [128, 8])
    MDTF = inp("MDTF", [128, NC_ * 48]); MDTOF = inp("MDTOF", [128, 48])

    OUT = nc.dram_tensor("OUT", [ITERS, 4 * OWN, 4 * W], F32, kind="ExternalOutput")

    with tile.TileContext(nc) as tc:
        with (
            tc.tile_pool(name="persist", bufs=1) as pp,
            tc.tile_pool(name="work", bufs=2) as wk,
            tc.tile_pool(name="stream", bufs=3) as st,
            tc.tile_pool(name="stgp", bufs=3) as stp,
            tc.tile_pool(name="agp", bufs=1) as agp,
            tc.tile_pool(name="psc", bufs=4, space="PSUM") as psc,
            tc.tile_pool(name="pst", bufs=4, space="PSUM") as pst,
            tc.tile_pool(name="dram", bufs=1, space="DRAM") as dr,
        ):
            # ---------- load persistent constants / weights ----------
            def load(t_dram, shape, dtype, name):
                t = pp.tile(list(shape), dtype, name=name, tag=name)
                if dtype in (BF16, F32R) and t_dram.dtype != dtype:
                    nc.gpsimd.dma_start(t[:], t_dram.ap())
                else:
                    nc.sync.dma_start(t[:], t_dram.ap())
                return t

            wenc0 = load(WENC0, (128, 9, 128), BF16, "wenc0")
            wenc1 = load(WENC1, (25, 9, 128), BF16, "wenc1")
            wz0 = load(WZ0, (128, 9, 128), BF16, "wz0"); wz1 = load(WZ1, (128, 9, 128), BF16, "wz1")
            wr0 = load(WR0, (128, 9, 128), BF16, "wr0"); wr1 = load(WR1, (128, 9, 128), BF16, "wr1")
            wq0 = load(WQ0, (128, 9, 128), BF16, "wq0"); wq1 = load(WQ1, (128, 9, 128), BF16, "wq1")
            wd1 = load(WD1, (128, 9, 128), BF16, "wd1"); wd2 = load(WD2, (128, 9, 75), BF16, "wd2")
            wm0 = load(WM0, (128, 9, 128), BF16, "wm0"); wm1 = load(WM1, (128, 9, 16), BF16, "wm1")
            benc = load(BENC, (128, 1), F32, "benc")
            bd1 = load(BD1, (128, 1), F32, "bd1"); bd2 = load(BD2, (75, 1), F32, "bd2")
            bz = load(BZ, (128, 1), F32, "bz"); br = load(BR, (128, 1), F32, "br"); bq = load(BQ, (128, 1), F32, "bq")
            bme0 = load(BME0, (128, 1), F32, "bme0"); bme1 = load(BME1, (16, 1), F32, "bme1")
            deltas = load(DELTAS, (128, 25), F32, "deltas")
            mrow = load(MROW, (128, SLAB), F32, "mrow")
            pxm = load(PXM, (128, NT), F32, "pxm")
            w0m = load(W0M, (128, 16), F32, "w0m"); w159m = load(W159M, (128, 16), F32, "w159m")
            agnt = load(AGNT, (128, 8), F32, "agnt"); agnb = load(AGNB, (128, 8), F32, "agnb")
            mdtf = load(MDTF, (128, NC_ * 48), F32, "mdtf")
            mdtof = load(MDTOF, (128, 48), F32, "mdtof")

            ident = pp.tile([128, 128], BF16, name="ident", tag="ident")
            ones128 = pp.tile([128, 128], BF16, name="ones128", tag="ones128")
            nc.vector.memset(ones128[:], 1.0)
            nc.gpsimd.affine_select(ident[:], ones128[:], pattern=[[-1, 128]], base=0,
                                    channel_multiplier=1, compare_op=OP.is_equal, fill=0.0)
            onesf = pp.tile([128, 128], F32, name="onesf", tag="onesf")
            nc.vector.memset(onesf[:], 1.0)
            identf = pp.tile([128, 128], F32, name="identf", tag="identf")
            nc.gpsimd.affine_select(identf[:], onesf[:], pattern=[[-1, 128]], base=0,
                                    channel_multiplier=1, compare_op=OP.is_equal, fill=0.0)

            # ---------- persistent state ----------
            volpb = pp.tile([128, NT, VW], BF16, name="volpb", tag="volpb")
            nc.vector.memset(volpb[:], 0.0)
            net = pp.tile([128, SLAB, WP], F32R, name="net", tag="net")
            netbf = pp.tile([128, SLAB, WP], BF16, name="netbf", tag="netbf")
            dtA = pp.tile([128, NT, 3], F32, name="dtA", tag="dtA")
            dtB = pp.tile([128, NT, 3], F32, name="dtB", tag="dtB")
            lc_all = pp.tile([128, 18, 75], BF16, name="lc_all", tag="lc_all")

            # ---------- Phase A: correlation volume ----------
            with tc.tile_pool(name="phA", bufs=1) as pA:
                volp = pA.tile([128, NT, 48], F32, name="volp", tag="volp")
                nc.vector.memset(volp[:], 0.0)
                Rg = dr.tile([SLAB, 160, 208], F32)  # reversed gram rows
                zk = wk.tile([128, 48], F32, name="zk", tag="zk")
                nc.vector.memset(zk[:], 0.0)
                for b in range(30):
                    nc.sync.dma_start(
                        bass.AP(tensor=Rg[:].tensor, offset=b * 128 * 208 + 160,
                                ap=[[208, 128], [1, 48]]), zk[:])
                for rh in range(2):
                    f1a = pA.tile([128, 2, 12, W], F32, name="f1a", tag="f1a")
                    f2a = pA.tile([128, 2, 12, W], F32, name="f2a", tag="f2a")
                    nc.sync.dma_start(f1a[:], F1.ap()[:, :, 12 * rh:12 * rh + 12, :].rearrange("c p r w -> p c r w"))
                    nc.sync.dma_start(f2a[:], F2.ap()[:, :, 12 * rh:12 * rh + 12, :].rearrange("c p r w -> p c r w"))
                    for r3 in range(4):
                        gs3 = pA.tile([128, 3, 2, W], F32, name="gs3", tag="gs3")
                        for j3 in range(3):
                            rr_ = r3 * 3 + j3
                            r = 12 * rh + rr_
                            for ci2, (w0, m) in enumerate(((0, 128), (128, 32))):
                                pg = pst.tile([128, W], F32, name="pg", tag="pstr")
                                for c in range(2):
                                    f2rc = f2a[:, c, rr_, :]
                                    rev = bass.AP(tensor=f2rc.tensor,
                                                  offset=f2rc.offset + 159,
                                                  ap=[list(f2rc.ap[0]), [-1, W]])
                                    nc.tensor.matmul(pg[:m, :], f1a[:, c, rr_, w0:w0 + m], rev,
                                                     start=(c == 0), stop=(c == 1))
                                nc.scalar.activation(gs3[:m, j3, ci2, :], pg[:m, :], AF.Copy, scale=1.0 / 16.0)
                        r0 = 12 * rh + r3 * 3
                        dst0 = bass.AP(tensor=Rg[:].tensor, offset=r0 * 33280,
                                       ap=[[208, 128], [33280, 3], [1, 160]])
                        nc.sync.dma_start(dst0, gs3[:, :, 0, :])
                        dst1 = bass.AP(tensor=Rg[:].tensor, offset=r0 * 33280 + 128 * 208,
                                       ap=[[208, 32], [33280, 3], [1, 160]])
                        nc.scalar.dma_start(dst1, gs3[0:32, :, 1, :])
                # diagonal extraction -> volp[:, t, VPAD:VPAD+48]
                rgf = Rg[:].flatten()
                for r in range(SLAB):
                    p0 = r * W
                    p1 = p0 + W
                    while p0 < p1:
                        t = p0 // 128
                        run = min(p1, (t + 1) * 128) - p0
                        w = p0 - r * W
                        src = bass.AP(tensor=rgf.tensor, offset=r * 33280 + w * 207 + 159,
                                      ap=[[207, run], [1, 48]])
                        deng = nc.sync if (p0 // 128) % 2 == 0 else nc.scalar
                        deng.dma_start(volp[p0 - 128 * t:p0 - 128 * t + run, t, :], src)
                        p0 += run
                # bf16 copy of the data region for the barrel (per-group, so it
                # pipelines with the diagonal-extraction DMAs)
                for g0 in range(0, NT, 6):
                    nc.gpsimd.tensor_copy(volpb[:, g0:g0 + 6, VPAD:VPAD + 48], volp[:, g0:g0 + 6, :])
                # initial top-3 (descending) of vol, masked by in-image
                for t in range(NT):
                    tv = wk.tile([128, 8], F32, name="tv", tag="tv")
                    ti = wk.tile([128, 8], U32, name="ti", tag="ti")
                    nc.vector.max(tv[:], volp[:, t, :])
                    nc.vector.max_index(ti[:], tv[:], volp[:, t, :])
                    tif = wk.tile([128, 3], F32, name="tif", tag="tif")
                    nc.vector.tensor_copy(tif[:], ti[:, 0:3].bitcast(I32))
                    nc.vector.tensor_scalar(dtA[:, t, :], tif[:], pxm[:, t:t + 1], None, op0=OP.mult)
            # net = tanh(net0); pads zero
            nc.vector.memset(net[:].bitcast(F32), 0.0)
            nc.gpsimd.dma_start(net[:, :, 1:161], NET0.ap())
            nc.scalar.activation(net[:, :, 1:161], net[:, :, 1:161], AF.Tanh)
            nc.vector.tensor_copy(netbf[:], net[:])

            dram_dispf = dr.tile([4224], F32)
            zk2 = wk.tile([128, 33], F32, name="zk2", tag="zk2")
            nc.vector.memset(zk2[:], 0.0)
            nc.sync.dma_start(
                bass.AP(tensor=dram_dispf[:].tensor, offset=0, ap=[[33, 128], [1, 33]]), zk2[:])
            agins, agouts, agdins, agdouts = [], [], [], []
            for _it in range(ITERS - 1):
                _ai = dr.tile([128, 1920], F32, name=f"agin{_it}", tag=f"agin{_it}")
                _ao = dr.tile([NC_ * 128, 1920], F32, addr_space="Shared", name=f"agout{_it}", tag=f"agout{_it}")
                _di = dr.tile([128, 48], F32, name=f"agdin{_it}", tag=f"agdin{_it}")
                _do = dr.tile([NC_ * 128, 48], F32, addr_space="Shared", name=f"agdout{_it}", tag=f"agdout{_it}")
                agins.append(_ai); agouts.append(_ao)
                agdins.append(_di); agdouts.append(_do)

            def conv_tiles(r_lo, r_hi):
                r = r_lo
                while r < r_hi:
                    nr = min(3, r_hi - r)
                    yield r, nr
                    r += nr

            def conv_layer(steps, r_lo, r_hi, drain, chunk=4, psum_p=128, border=None):
                # steps: list of (w_tile, src_tile, row_offset); each expands
                # to 9 shifts. s-outer: one weight load serves all blocks.
                blocks = list(conv_tiles(r_lo, r_hi))
                if border is not None:
                    blocks = [b for b in blocks if b[0] in border] + \
                             [b for b in blocks if b[0] not in border]
                flat = [(wt, s, src, roff) for (wt, src, roff) in steps for s in range(9)]
                nstep = len(flat)
                for i0 in range(0, len(blocks), chunk):
                    blk = blocks[i0:i0 + chunk]
                    pss = [psc.tile([psum_p, nr, W], F32, name=f"ps{r0}", tag="pscv")
                           for (r0, nr) in blk]
                    for si, (wt, s, src, roff) in enumerate(flat):
                        dy, dx = s // 3, s % 3
                        for bi2, (r0, nr) in enumerate(blk):
                            nc.tensor.matmul(pss[bi2][:], wt[:, s, :],
                                             _conv_shift_rhs(src, r0 + roff, dy, dx, nr),
                                             start=(si == 0), stop=(si == nstep - 1))
                    for bi2, (r0, nr) in enumerate(blk):
                        drain(pss[bi2], r0, nr)

            dt_cur, dt_nxt = dtA, dtB
            for it in range(ITERS):
              with tc.tile_pool(name=f"itE{it}", bufs=1) as itE:
                  corrA = itE.tile([128, SLAB, WP], BF16, name="corrA", tag="corrA")
                  corrB = itE.tile([25, SLAB, WP], BF16, name="corrB", tag="corrB")
                  cpx_all = itE.tile([128, NT, 160], BF16, name="cpx_all", tag="cpx_all")
                  nc.vector.memset(corrA[:], 0.0)
                  nc.vector.memset(corrB[:], 0.0)
                  nc.vector.memset(cpx_all[:, :, 153:160], 0.0)

                  # ---------- B1: barrel gather; masks in 3 tile-range batches ----------
                  # iteration 0: dt in [0,47] so off-24 in [0,47] -> 6 stages, base col 24
                  BITS_IT = BITS[1:] if it == 0 else BITS
                  BC = 24 if it == 0 else 0
                  IW = 115 if it == 0 else 179
                  mbs = itE.tile([128, 7, NK], U8, name="mbs", tag="mbs")
                  avb = itE.tile([128, NK], BF16, name="avb", tag="avb")
                  bvb = itE.tile([128, NK], BF16, name="bvb", tag="bvb")
                  for (lo_t, hi_t) in ((12, 18), (0, 12), (18, 30)):
                      nk = (hi_t - lo_t) * 3
                      c0 = lo_t * 3
                      dts = dt_cur[:, lo_t:hi_t, :].rearrange("p t k -> p (t k)")
                      offa = wk.tile([128, nk], F32, name="offa", tag="offa")
                      nc.vector.tensor_scalar(offa[:], dts, 24.0 - BC, None, op0=OP.add)
                      nc.vector.tensor_scalar(offa[:], offa[:], 0.0, 96.0, op0=OP.max, op1=OP.min)
                      rem = offa
                      for bi, bit in enumerate(BITS_IT):
                          mf = wk.tile([128, nk], F32, name="mf", tag=f"mf{bi % 2}")
                          nc.vector.tensor_scalar(mf[:], rem[:], float(bit), None, op0=OP.is_ge)
                          nc.vector.scalar_tensor_tensor(rem[:], mf[:], float(-bit), rem[:],
                                                         op0=OP.mult, op1=OP.add)
                          nc.scalar.copy(mbs[:, bi, c0:c0 + nk], mf[:])
                      dti = wk.tile([128, nk], I32, name="dti", tag="dti")
                      nc.vector.tensor_copy(dti[:], dts)
                      nc.vector.tensor_scalar(dti[:], dti[:], 1, None, op0=OP.bitwise_and)
                      parf = wk.tile([128, nk], F32, name="parf", tag="parf")
                      nc.vector.tensor_copy(parf[:], dti[:])
                      nc.vector.tensor_scalar(avb[:, c0:c0 + nk], parf[:], 0.25, None, op0=OP.mult)
                      nc.vector.tensor_scalar(bvb[:, c0:c0 + nk], parf[:], -0.25, 0.5, op0=OP.mult, op1=OP.add)
                      nc.vector.tensor_copy(cpx_all[:, lo_t:hi_t, 150:153], dt_cur[:, lo_t:hi_t, :])
                  TC = 6
                  with tc.tile_pool(name=f"bp{it}", bufs=1) as bpool:
                      bb = [bpool.tile([128, TC, 3, 179], BF16, name=f"bb{j}", tag=f"bb{j}")
                            for j in range(3)]
                      t1 = bpool.tile([128, TC, 3, 25], BF16, name="t1", tag="t1")
                      t2 = bpool.tile([128, TC, 3, 25], BF16, name="t2", tag="t2")
                      CHUNKS = [(12, 0), (0, 1), (6, 2), (18, 0), (24, 1)]
                      for (t0, bj) in CHUNKS:
                          tn = TC
                          buf = bb[bj]
                          nc.vector.tensor_copy(
                              buf[:, 0:tn, :, 0:IW],
                              volpb[:, t0:t0 + tn, BC:BC + IW].unsqueeze(2).broadcast_to([128, tn, 3, IW]))
                          for bi, bit in enumerate(BITS_IT):
                              wd = BWID[bit]
                              mbv = mbs[:, bi, 3 * t0:3 * (t0 + tn)] \
                                  .rearrange("p (t k) -> p t k", k=3).unsqueeze(3) \
                                  .broadcast_to([128, tn, 3, wd])
                              nc.vector.copy_predicated(buf[:, 0:tn, :, 0:wd], mbv,
                                                        buf[:, 0:tn, :, bit:bit + wd])
                          g = buf[:, 0:tn, :, 0:52]
                          for k in range(3):
                              nc.scalar.copy(cpx_all[:, t0:t0 + tn, 50 * k:50 * k + 25],
                                             g[:, :, k, 13:38])
                          lo = max(6, t0); hi = min(24, t0 + tn)
                          if lo < hi:
                              nc.scalar.copy(
                                  lc_all[:, lo - 6:hi - 6, :].rearrange("p t (k c) -> p t k c", k=3),
                                  g[:, lo - t0:hi - t0, :, 13:38])
                          nc.gpsimd.tensor_tensor(t1[:, 0:tn], g[:, :, :, 0:49:2], g[:, :, :, 3:52:2], op=OP.add)
                          nc.gpsimd.tensor_tensor(t2[:, 0:tn], g[:, :, :, 1:50:2], g[:, :, :, 2:51:2], op=OP.add)
                          nc.gpsimd.tensor_tensor(
                              t1[:, 0:tn], t1[:, 0:tn],
                              avb[:].rearrange("p (t k) -> p t k", k=3)[:, t0:t0 + tn, :]
                                  .unsqueeze(3).broadcast_to([128, tn, 3, 25]),
                              op=OP.mult)
                          nc.gpsimd.tensor_tensor(
                              t2[:, 0:tn], t2[:, 0:tn],
                              bvb[:].rearrange("p (t k) -> p t k", k=3)[:, t0:t0 + tn, :]
                                  .unsqueeze(3).broadcast_to([128, tn, 3, 25]),
                              op=OP.mult)
                          for k in range(3):
                              nc.gpsimd.tensor_tensor(cpx_all[:, t0:t0 + tn, 50 * k + 25:50 * k + 50],
                                                      t1[:, 0:tn, k, :], t2[:, 0:tn, k, :], op=OP.add)

                  # ---------- B2: transpose to spatial ----------
                  for t in range(NT):
                      pa = pst.tile([128, 128], BF16, name="pa", tag="pstr")
                      pb = pst.tile([32, 128], BF16, name="pb", tag="pstr")
                      nc.tensor.transpose(pa[:], cpx_all[:, t, 0:128], ident[:])
                      nc.tensor.transpose(pb[:], cpx_all[:, t, 128:160], ident[:])
                      p0 = t * 128
                      while p0 < (t + 1) * 128:
                          r = p0 // W
                          run = min((t + 1) * 128, (r + 1) * W) - p0
                          w0 = p0 - r * W
                          c0 = p0 - t * 128
                          nc.scalar.copy(corrA[:, r, 1 + w0:1 + w0 + run], pa[:, c0:c0 + run])
                          nc.scalar.copy(corrB[:, r, 1 + w0:1 + w0 + run], pb[0:25, c0:c0 + run])
                          p0 += run
                  # ---------- B3: conv stack (s-outer, weight-reload amortized) ----------
                  xbf = itE.tile([128, SLAB, WP], BF16, name="xbf", tag="xbf")
                  nc.vector.memset(xbf[:], 0.0)

                  def drain_enc(ps, r0, nr):
                      nc.scalar.activation(xbf[:, r0:r0 + nr, 1:161], ps[:], AF.Relu, bias=benc[:])
                  conv_layer([(wenc0, corrA[:], 0), (wenc1, corrB[:], 0)], 1, 23, drain_enc)
                  nc.vector.tensor_tensor(xbf[:, 1:23, :], xbf[:, 1:23, :],
                                          mrow[:, 1:23].unsqueeze(2).broadcast_to([128, 22, WP]), op=OP.mult)

                  # r conv first (rows [2,22)) -> rnet
                  rnet = itE.tile([128, SLAB, WP], BF16, name="rnet", tag="rnet")
                  nc.vector.memset(rnet[:], 0.0)

                  def drain_r(ps, r0, nr):
                      cs = st.tile([128, 3, W], F32, name="csr", tag="cs")
                      nc.sync.dma_start(cs[:, 0:nr, :], CRS.ap()[:, r0 - 2:r0 - 2 + nr, :])
                      tadd = wk.tile([128, nr, W], F32, name="tar", tag="tadd")
                      nc.vector.tensor_tensor(tadd[:], ps[:], cs[:, 0:nr, :], op=OP.add)
                      rs = wk.tile([128, nr, W], F32, name="rs", tag="qs")
                      nc.scalar.activation(rs[:], tadd[:], AF.Sigmoid, bias=br[:])
                      nc.vector.tensor_tensor(rnet[:, r0:r0 + nr, 1:161], rs[:], net[:, r0:r0 + nr, 1:161], op=OP.mult)
                  conv_layer([(wr0, netbf[:], 0), (wr1, xbf[:], 0)], 2, 22, drain_r)

                  # z conv (rows [3,21)) -> zs_all
                  zs_all = itE.tile([128, 18, W], BF16, name="zs_all", tag="zs_all")

                  def drain_z(ps, r0, nr):
                      cs = st.tile([128, 3, W], F32, name="csz", tag="cs")
                      nc.sync.dma_start(cs[:, 0:nr, :], CZS.ap()[:, r0 - 2:r0 - 2 + nr, :])
                      tz = wk.tile([128, nr, W], F32, name="tz", tag="tadd")
                      nc.vector.tensor_tensor(tz[:], ps[:], cs[:, 0:nr, :], op=OP.add)
                      nc.scalar.activation(zs_all[:, r0 - 3:r0 - 3 + nr, :], tz[:], AF.Sigmoid, bias=bz[:])
                  conv_layer([(wz0, netbf[:], 0), (wz1, xbf[:], 0)], 3, 21, drain_z)

                  # q conv + net update (rows [3,21))
                  def drain_q(ps, r0, nr):
                      cs = st.tile([128, 3, W], F32, name="csq", tag="cs")
                      nc.sync.dma_start(cs[:, 0:nr, :], CQS.ap()[:, r0 - 2:r0 - 2 + nr, :])
                      tadd = wk.tile([128, nr, W], F32, name="taq", tag="tadd")
                      nc.vector.tensor_tensor(tadd[:], ps[:], cs[:, 0:nr, :], op=OP.add)
                      qs = wk.tile([128, nr, W], F32, name="qs", tag="qs")
                      nc.scalar.activation(qs[:], tadd[:], AF.Tanh, bias=bq[:])
                      dqn = wk.tile([128, nr, W], F32, name="dqn", tag="dqn")
                      nc.vector.tensor_tensor(dqn[:], qs[:], net[:, r0:r0 + nr, 1:161], op=OP.subtract)
                      nc.vector.tensor_tensor(dqn[:], dqn[:], zs_all[:, r0 - 3:r0 - 3 + nr, :], op=OP.mult)
                      nc.vector.tensor_tensor(net[:, r0:r0 + nr, 1:161], net[:, r0:r0 + nr, 1:161], dqn[:], op=OP.add)
                  conv_layer([(wq0, rnet[:], 0), (wq1, xbf[:], 0)], 3, 21, drain_q,
                             border=(6, 9, 12, 15))
                  nc.vector.tensor_tensor(net[:, 6:18, :], net[:, 6:18, :],
                                          mrow[:, 6:18].unsqueeze(2).broadcast_to([128, 12, WP]), op=OP.mult)
                  if it < ITERS - 1:
                      agin = agins[it]
                      nc.sync.dma_start(agin[:, 0:960].rearrange("p (a b) -> p a b", a=6), net[:, 6:12, 1:161].bitcast(F32))
                      nc.sync.dma_start(agin[:, 960:1920].rearrange("p (a b) -> p a b", a=6), net[:, 12:18, 1:161].bitcast(F32))
                      nc.gpsimd.collective_compute(
                          "AllGather", OP.bypass, replica_groups=[list(range(NC_))],
                          ins=[agin[:].opt()], outs=[agouts[it][:].opt()])
                  nc.vector.tensor_copy(netbf[:, 6:18, :], net[:, 6:18, :])
                  nc.vector.tensor_tensor(net[:, 3:6, :], net[:, 3:6, :],
                                          mrow[:, 3:6].unsqueeze(2).broadcast_to([128, 3, WP]), op=OP.mult)
                  nc.vector.tensor_tensor(net[:, 18:21, :], net[:, 18:21, :],
                                          mrow[:, 18:21].unsqueeze(2).broadcast_to([128, 3, WP]), op=OP.mult)
                  nc.vector.tensor_copy(netbf[:, 3:6, :], net[:, 3:6, :])
                  nc.vector.tensor_copy(netbf[:, 18:21, :], net[:, 18:21, :])

              with tc.tile_pool(name=f"itL{it}", bufs=1) as itL:
                  # d1 (bf16), dlc=d2, me=m-conv
                  d1sl = itL.tile([128, 16, WP], BF16, name="d1sl", tag="d1sl")
                  nc.vector.memset(d1sl[:], 0.0)

                  def drain_d1(ps, r0, nr):
                      nc.scalar.activation(d1sl[:, r0 - 4:r0 - 4 + nr, 1:161], ps[:], AF.Relu, bias=bd1[:])
                  conv_layer([(wd1, netbf[:], 0)], 4, 20, drain_d1)
                  nc.vector.tensor_tensor(d1sl[:], d1sl[:],
                                          mrow[:, 4:20].unsqueeze(2).broadcast_to([128, 16, WP]), op=OP.mult)

                  gsp = itL.tile([75, 2304], F32, name="gsp", tag="gsp")
                  mes0t = itL.tile([128, 2304], F32, name="mes0t", tag="mes0t")
                  mes1t = itL.tile([16, 2304], F32, name="mes1t", tag="mes1t")
                  dlcsp = gsp[:, :]
                  mesp1 = mes1t[:, :]
                  mesp0 = mes0t[:, :]
                  nc.vector.memset(gsp[:], 0.0)

                  def drain_d2(ps, r0, nr):
                      col = r0 * W - 768
                      nc.scalar.activation(dlcsp[:, col:col + nr * W],
                                           ps[:].rearrange("p a b -> p (a b)"),
                                           AF.Identity, bias=bd2[:])

                  def drain_m0(ps, r0, nr):
                      col = r0 * W - 768
                      nc.scalar.activation(mesp0[:, col:col + nr * W],
                                           ps[:].rearrange("p a b -> p (a b)"),
                                           AF.Exp, bias=bme0[:], scale=0.25)

                  def drain_m1(ps, r0, nr):
                      col = r0 * W - 768
                      nc.scalar.activation(mesp1[:, col:col + nr * W],
                                           ps[:].rearrange("p a b -> p (a b)"),
                                           AF.Exp, bias=bme1[:], scale=0.25)
                  conv_layer([(wd2, d1sl[:], -4)], 5, 19, drain_d2, psum_p=75)


                  # ---------- B4: prob / disp / top-3, groups of 6 tiles ----------
                  for grp in range(3):
                      t0 = 6 + grp * 6
                      pdg = pst.tile([128, 6, 75], F32, name="pdg", tag="pstr")
                      for j in range(6):
                          col = (t0 - 6 + j) * 128
                          nc.tensor.transpose(pdg[:, j, :], dlcsp[:, col:col + 128], identf[0:75, 0:75])
                      eg = wk.tile([128, 6, 75], F32, name="eg", tag="eg")
                      nc.vector.tensor_tensor(eg[:], pdg[:], lc_all[:, t0 - 6:t0, :], op=OP.add)
                      nc.scalar.activation(eg[:], eg[:], AF.Exp)
                      Ek = wk.tile([128, 6, 3], F32, name="Ek", tag="Ek")
                      nc.vector.reduce_sum(Ek[:], eg[:].rearrange("p t (k c) -> p t k c", k=3),
                                           axis=mybir.AxisListType.X)
                      ssum = wk.tile([128, 6], F32, name="ssum", tag="ssum")
                      nc.vector.reduce_sum(ssum[:], Ek[:], axis=mybir.AxisListType.X)
                      srec = wk.tile([128, 6], F32, name="srec", tag="srec")
                      nc.vector.reciprocal(srec[:], ssum[:])
                      es = wk.tile([128, 6, 75], F32, name="es", tag="es")
                      nc.vector.tensor_tensor(
                          es[:].rearrange("p t (k c) -> p t k c", k=3),
                          eg[:].rearrange("p t (k c) -> p t k c", k=3),
                          deltas[:].unsqueeze(1).unsqueeze(1).broadcast_to([128, 6, 3, 25]), op=OP.mult)
                      esum = wk.tile([128, 6], F32, name="esum", tag="esum")
                      nc.vector.reduce_sum(esum[:], es[:], axis=mybir.AxisListType.X)
                      dtE = wk.tile([128, 6, 3], F32, name="dtE", tag="dtE")
                      nc.vector.tensor_tensor(dtE[:], Ek[:], dt_cur[:, t0:t0 + 6, :], op=OP.mult)
                      desum = wk.tile([128, 6], F32, name="desum", tag="desum")
                      nc.vector.reduce_sum(desum[:], dtE[:], axis=mybir.AxisListType.X)
                      disp = wk.tile([128, 6], F32, name="disp", tag="disp")
                      nc.vector.tensor_tensor(disp[:], esum[:], desum[:], op=OP.add)
                      nc.vector.tensor_tensor(disp[:], disp[:], srec[:], op=OP.mult)
                      nc.vector.tensor_tensor(disp[:], disp[:], pxm[:, t0:t0 + 6], op=OP.mult)
                      nc.vector.tensor_scalar(disp[:], disp[:], 4.0, None, op0=OP.mult)
                      nc.sync.dma_start(
                          bass.AP(tensor=dram_dispf[:].tensor, offset=t0 * 128, ap=[[1, 128], [128, 6]]),
                          disp[:])
                      if it < ITERS - 1:
                          tif_g = wk.tile([128, 6, 3], F32, name="tifg", tag="tifg")
                          for j in range(6):
                              tv = wk.tile([128, 8], F32, name="tv2", tag="tv2")
                              ti = wk.tile([128, 8], U32, name="ti2", tag="ti2")
                              nc.vector.max(tv[:], eg[:, j, :])
                              nc.vector.max_index(ti[:], tv[:], eg[:, j, :])
                              nc.vector.tensor_copy(tif_g[:, j, :], ti[:, 0:3].bitcast(I32))
                          s1 = wk.tile([128, 6, 3], F32, name="s1", tag="s1")
                          s2 = wk.tile([128, 6, 3], F32, name="s2", tag="s2")
                          nc.vector.tensor_scalar(s1[:], tif_g[:], 25.0, None, op0=OP.is_ge)
                          nc.vector.tensor_scalar(s2[:], tif_g[:], 50.0, None, op0=OP.is_ge)
                          jv = wk.tile([128, 6, 3], F32, name="jv", tag="jv")
                          nc.vector.scalar_tensor_tensor(jv[:], s1[:], -25.0, tif_g[:], op0=OP.mult, op1=OP.add)
                          nc.vector.scalar_tensor_tensor(jv[:], s2[:], -25.0, jv[:], op0=OP.mult, op1=OP.add)
                          d10 = wk.tile([128, 6, 1], F32, name="d10", tag="d10")
                          d21 = wk.tile([128, 6, 1], F32, name="d21", tag="d21")
                          nc.vector.tensor_tensor(d10[:], dt_cur[:, t0:t0 + 6, 1:2], dt_cur[:, t0:t0 + 6, 0:1], op=OP.subtract)
                          nc.vector.tensor_tensor(d21[:], dt_cur[:, t0:t0 + 6, 2:3], dt_cur[:, t0:t0 + 6, 1:2], op=OP.subtract)
                          v = wk.tile([128, 6, 3], F32, name="v", tag="v")
                          nc.vector.tensor_tensor(v[:], s2[:], d21[:].broadcast_to([128, 6, 3]), op=OP.mult)
                          nc.vector.tensor_tensor(v[:], v[:], dt_cur[:, t0:t0 + 6, 0:1].broadcast_to([128, 6, 3]), op=OP.add)
                          tmpv = wk.tile([128, 6, 3], F32, name="tmpv", tag="tmpv")
                          nc.vector.tensor_tensor(tmpv[:], s1[:], d10[:].broadcast_to([128, 6, 3]), op=OP.mult)
                          nc.vector.tensor_tensor(v[:], v[:], tmpv[:], op=OP.add)
                          nc.vector.tensor_tensor(v[:], v[:], jv[:], op=OP.add)
                          nc.vector.tensor_scalar(dt_nxt[:, t0:t0 + 6, :], v[:], -12.0, None, op0=OP.add)

                  conv_layer([(wm0, netbf[:], 0)], 5, 19, drain_m0)
                  conv_layer([(wm1, netbf[:], 0)], 5, 19, drain_m1, psum_p=16)

                  # ---------- B6a: dt AllGather + net stage loads ----------
                  stgs = []
                  if it < ITERS - 1:
                      agdin = agdins[it]
                      nc.sync.dma_start(agdin[:, 0:48].rearrange("p (a b) -> p a b", a=16), dt_nxt[:, 7:23, :])
                      nc.gpsimd.collective_compute(
                          "AllGather", OP.bypass, replica_groups=[list(range(NC_))],
                          ins=[agdin[:].opt()], outs=[agdouts[it][:].opt()])
                      for rr in range(NC_):
                          stg = stp.tile([128, 1920], BF16, name=f"stg{rr}", tag="stg")
                          nc.gpsimd.dma_start(stg[:], agouts[it][:][rr * 128:(rr + 1) * 128, :])
                          stgs.append(stg)

                  # ---------- B5: upsample own rows ----------
                  Dall = itL.tile([128, 16, 9], F32, name="Dall", tag="Dall")
                  for jblk in range(3):
                      srcoff = 7 * 128 + (jblk - 1) * 160 - 1
                      nc.scalar.dma_start(
                          Dall[:, :, 3 * jblk:3 * jblk + 3],
                          bass.AP(tensor=dram_dispf[:].tensor, offset=srcoff,
                                  ap=[[1, 128], [128, 16], [1, 3]]))
                  for qq in (0, 3, 6):
                      nc.vector.tensor_tensor(Dall[:, :, qq], Dall[:, :, qq], w0m[:], op=OP.mult)
                  for qq in (2, 5, 8):
                      nc.vector.tensor_tensor(Dall[:, :, qq], Dall[:, :, qq], w159m[:], op=OP.mult)
                  met_all = itL.tile([128, 16, 144], F32, name="met_all", tag="met_all")
                  for t in range(7, 23):
                      col = t * 128 - 768
                      pm0 = pst.tile([128, 128], F32, name="pm0", tag="pstr")
                      pm1 = pst.tile([128, 16], F32, name="pm1", tag="pstr")
                      nc.tensor.transpose(pm0[:], mesp0[:, col:col + 128], identf[:])
                      nc.tensor.transpose(pm1[:], mesp1[:, col:col + 128], identf[0:16, 0:16])
                      nc.scalar.copy(met_all[:, t - 7, 0:128], pm0[:])
                      nc.scalar.copy(met_all[:, t - 7, 128:144], pm1[:])
                  msum = wk.tile([128, 16, 16], F32, name="msum", tag="msum")
                  nc.vector.reduce_sum(msum[:], met_all[:].rearrange("p t (q f) -> p t f q", q=9),
                                       axis=mybir.AxisListType.X)
                  mrec = wk.tile([128, 16, 16], F32, name="mrec", tag="mrec")
                  nc.vector.reciprocal(mrec[:], msum[:])
                  acc = wk.tile([128, 16, 16], F32, name="acc", tag="acc")
                  tmpm = wk.tile([128, 16, 16], F32, name="tmpm", tag="tmpm")
                  nc.vector.tensor_tensor(
                      acc[:], met_all[:, :, 0:16],
                      Dall[:, :, 0].unsqueeze(2).broadcast_to([128, 16, 16]), op=OP.mult)
                  for qq in range(1, 9):
                      nc.vector.tensor_tensor(
                          tmpm[:], met_all[:, :, 16 * qq:16 * qq + 16],
                          Dall[:, :, qq].unsqueeze(2).broadcast_to([128, 16, 16]), op=OP.mult)
                      nc.vector.tensor_tensor(acc[:], acc[:], tmpm[:], op=OP.add)
                  pred = wk.tile([128, 16, 16], F32, name="pred", tag="pred")
                  nc.vector.tensor_tensor(pred[:], acc[:], mrec[:], op=OP.mult)
                  for t in range(7, 23):
                      p0 = t * 128
                      while p0 < (t + 1) * 128:
                          r = p0 // W
                          run = min((t + 1) * 128, (r + 1) * W) - p0
                          w0 = p0 - r * W
                          if 6 <= r < 18:
                              dst = bass.AP(tensor=OUT.ap().tensor,
                                            offset=it * 48 * 640 + (r - 6) * 4 * 640 + w0 * 4,
                                            ap=[[4, run], [640, 4], [1, 4]])
                              nc.sync.dma_start(dst, pred[p0 - t * 128:p0 - t * 128 + run, t - 7, :].rearrange("p (a b) -> p a b", a=4))
                          p0 += run

                  # ---------- B6b: consume AllGathers ----------
                  if it < ITERS - 1:
                      vt = agp.tile([128, 960], F32, name="vt", tag="vt")
                      vb = agp.tile([128, 960], F32, name="vb", tag="vb")
                      dtv = agp.tile([128, 48], F32, name="dtv", tag="dtv")
                      nc.vector.memset(vt[:], 0.0)
                      nc.vector.memset(vb[:], 0.0)
                      nc.vector.memset(dtv[:], 0.0)
                      dttmp = wk.tile([128, 48], F32, name="dttmp", tag="dttmp")
                      stgd_all = stp.tile([128, NC_, 48], F32, name="stgd_all", tag="stgd_all")
                      nc.sync.dma_start(
                          stgd_all[:],
                          bass.AP(tensor=agdouts[it][:].tensor, offset=0,
                                  ap=[[48, 128], [128 * 48, NC_], [1, 48]]))
                      for rr in range(NC_):
                          stg = stgs[rr]
                          nc.vector.scalar_tensor_tensor(vt[:], stg[:, 960:1920], agnt[:, rr:rr + 1], vt[:], op0=OP.mult, op1=OP.add)
                          nc.vector.scalar_tensor_tensor(vb[:], stg[:, 0:960], agnb[:, rr:rr + 1], vb[:], op0=OP.mult, op1=OP.add)
                          nc.vector.tensor_tensor(dttmp[:], stgd_all[:, rr, :],
                                                  mdtf[:, rr * 48:(rr + 1) * 48], op=OP.mult)
                          nc.vector.tensor_tensor(dtv[:], dtv[:], dttmp[:], op=OP.add)
                      nc.vector.tensor_copy(net[:, 0:6, 1:161], vt[:].rearrange("p (a b) -> p a b", a=6))
                      nc.vector.tensor_copy(net[:, 18:24, 1:161], vb[:].rearrange("p (a b) -> p a b", a=6))
                      nc.vector.tensor_copy(netbf[:, 0:6, :], net[:, 0:6, :])
                      nc.vector.tensor_copy(netbf[:, 18:24, :], net[:, 18:24, :])
                      topv = dt_nxt[:, 0:8, :].rearrange("p t k -> p (t k)")
                      botv = dt_nxt[:, 22:30, :].rearrange("p t k -> p (t k)")
                      nc.vector.tensor_tensor(topv, topv, mdtof[:, 24:48], op=OP.mult)
                      nc.vector.tensor_tensor(topv, topv, dtv[:, 24:48], op=OP.add)
                      nc.vector.tensor_tensor(botv, botv, mdtof[:, 0:24], op=OP.mult)
                      nc.vector.tensor_tensor(botv, botv, dtv[:, 0:24], op=OP.add)
              dt_cur, dt_nxt = dt_nxt, dt_cur

    nc.compile()
    return nc


def _prep_inputs(full):
    bf = ml_dtypes.bfloat16
    def lhsT(warr, kslice, mslice):
        # [out,in,3,3] -> [K, 9, M]
        w = warr[mslice, kslice]
        return np.ascontiguousarray(w.transpose(1, 2, 3, 0).reshape(w.shape[1], 9, w.shape[0]))

    enc_w = full['enc_w']; m_w = full['m_w']
    shared = {
        'WENC0': lhsT(enc_w, slice(0, 128), slice(None)).astype(bf),
        'WENC1': lhsT(enc_w, slice(128, 153), slice(None)).astype(bf),
        'WZ0': lhsT(full['gz_w'], slice(0, 128), slice(None)).astype(bf),
        'WZ1': lhsT(full['gz_w'], slice(128, 256), slice(None)).astype(bf),
        'WR0': lhsT(full['gr_w'], slice(0, 128), slice(None)).astype(bf),
        'WR1': lhsT(full['gr_w'], slice(128, 256), slice(None)).astype(bf),
        'WQ0': lhsT(full['gq_w'], slice(0, 128), slice(None)).astype(bf),
        'WQ1': lhsT(full['gq_w'], slice(128, 256), slice(None)).astype(bf),
        'WD1': lhsT(full['d1_w'], slice(None), slice(None)).astype(np.float32),
        'WD2': lhsT(full['d2_w'], slice(None), slice(None)).astype(np.float32),
        'WM0': lhsT(m_w, slice(None), slice(0, 128)).astype(bf),
        'WM1': lhsT(m_w, slice(None), slice(128, 144)).astype(bf),
        'BENC': full['enc_b'].reshape(128, 1).astype(np.float32),
        'BD1': full['d1_b'].reshape(128, 1).astype(np.float32),
        'BD2': full['d2_b'].reshape(75, 1).astype(np.float32),
        'BZ': full['gz_b'].reshape(128, 1).astype(np.float32),
        'BR': full['gr_b'].reshape(128, 1).astype(np.float32),
        'BQ': full['gq_b'].reshape(128, 1).astype(np.float32),
        'BME0': (0.25 * full['m_b'][0:128]).reshape(128, 1).astype(np.float32),
        'BME1': (0.25 * full['m_b'][128:144]).reshape(16, 1).astype(np.float32),
        'DELTAS': np.tile(np.arange(-RT, RT + 1, dtype=np.float32), (128, 1)),
    }
    in_maps = []
    for i in range(NC_):
        R0 = 12 * i - HALO
        rows = np.arange(R0, R0 + SLAB)
        inimg = ((rows >= 0) & (rows < H)).astype(np.float32)

        def slab(x, lo=0, hi=SLAB):
            out = np.zeros((x.shape[0], hi - lo, W), np.float32)
            for j in range(lo, hi):
                r = R0 + j
                if 0 <= r < H:
                    out[:, j - lo] = x[:, r]
            return out

        px_in = np.repeat(inimg, W)
        pxm = np.zeros((128, NT), np.float32)
        for t in range(NT):
            pxm[:, t] = px_in[t * 128:(t + 1) * 128]
        w0m = np.zeros((128, 16), np.float32); w159m = np.zeros((128, 16), np.float32)
        for t in range(7, 23):
            pxs = np.arange(t * 128, (t + 1) * 128)
            w0m[:, t - 7] = (pxs % W != 0).astype(np.float32)
            w159m[:, t - 7] = (pxs % W != W - 1).astype(np.float32)
        agnt = np.zeros((128, 8), np.float32); agnb = np.zeros((128, 8), np.float32)
        if i - 1 >= 0:
            agnt[:, i - 1] = 1.0
        if i + 1 < NC_:
            agnb[:, i + 1] = 1.0
        # batched dt-halo masks: dtv col j (0..47) maps to sender packed tile
        # sp = 7 + j//3; bottom tiles (22..29) come from rank i+1 (sp 7..14),
        # top tiles (0..7) from rank i-1 (sp 15..22).
        mdtf = np.zeros((128, NC_ * 48), np.float32)
        mdtof = np.zeros((128, 48), np.float32)
        for kk in range(8):  # top tiles k = kk
            pxs = np.arange(kk * 128, (kk + 1) * 128)
            halo = (pxs < 6 * W).astype(np.float32) * px_in[pxs.clip(0, NPX - 1)]
            own = (pxs >= 6 * W).astype(np.float32)
            if i - 1 >= 0:
                for c in range(3):
                    mdtf[:, (i - 1) * 48 + 24 + 3 * kk + c] = halo
            for c in range(3):
                mdtof[:, 24 + 3 * kk + c] = own
        for kk in range(8):  # bottom tiles k = kk + 22
            k = kk + 22
            pxs = np.arange(k * 128, (k + 1) * 128)
            halo = (pxs >= 18 * W).astype(np.float32) * px_in[pxs.clip(0, NPX - 1)]
            own = (pxs < 18 * W).astype(np.float32)
            if i + 1 < NC_:
                for c in range(3):
                    mdtf[:, (i + 1) * 48 + 3 * kk + c] = halo
            for c in range(3):
                mdtof[:, 3 * kk + c] = own
        m = dict(shared)
        m.update({
            'F1': slab(full['fmap1'][0]).reshape(2, 128, SLAB, W),
            'F2': slab(full['fmap2'][0]).reshape(2, 128, SLAB, W),
            'NET0': slab(full['net0'][0]),
            'CZS': slab(full['cz'][0], 2, 22), 'CQS': slab(full['cq'][0], 2, 22),
            'CRS': slab(full['cr'][0], 2, 22),
            'MROW': np.tile(inimg, (128, 1)),
            'PXM': pxm, 'W0M': w0m, 'W159M': w159m,
            'AGNT': agnt, 'AGNB': agnb,
            'MDTF': mdtf, 'MDTOF': mdtof,
        })
        in_maps.append(m)
    return in_maps


def kernel(**inputs):
    assert int(inputs['iters']) == ITERS
    if 'nc' not in _cache:
        _cache['nc'] = build()
    full = {k: np.asarray(v) for k, v in inputs.items()}
    in_maps = _prep_inputs(full)
    res = bass_utils.run_bass_kernel_spmd(_cache['nc'], in_maps, core_ids=list(range(NC_)))
    global _last_res
    _last_res = res
    out = np.zeros((ITERS, 1, 1, 4 * H, 4 * W), np.float32)
    for i in range(NC_):
        out[:, 0, 0, 48 * i:48 * i + 48, :] = res.results[i]['OUT']
    return out
